# revision 1
# baseline (speedup 1.0000x reference)
"""Trainium2 Bass kernel for a 6-layer GPT (D=512, H=8, T=1024, B=2, V=50257).

Strategy (8 NeuronCores):
- Token-shard the transformer body: core c owns 256 tokens (cores 0-3 =
  batch 0 chunks 0-3, cores 4-7 = batch 1 chunks 0-3).
- Per layer: LN1 -> QKV (q/k in transposed [dim, tok] layout, v natural)
  -> AllGather K,V across the 4-core batch group -> causal attention in
  transposed layout (softmax denominator fused as a ones-row of V; no
  on-chip transposes in the attention loop) -> Wo + residual -> LN2 ->
  MLP (mm1 transposed so mm2 needs no transpose) -> residual.
- Final LN -> AllGather hidden state across all 8 cores -> vocab-sharded
  LM head: core c computes logits[:, c*6656:(c+1)*6656] (Wlm zero-padded
  to 8*6656 columns host-side).
- Host folds LN gamma/beta and the 1/sqrt(HS) score scale into the
  weights; embedding gather happens host-side (tiny).
"""

import numpy as np

import concourse.bass as bass
import concourse.tile as tile
from concourse import bacc, mybir
from concourse import bass_utils
from concourse.bass import ds, ts
from concourse.masks import make_identity

FP = mybir.dt.float32
AF = mybir.ActivationFunctionType
OP = mybir.AluOpType

V, D, T, L, H, HS, B = 50257, 512, 1024, 6, 8, 64, 2
FF = 4 * D
EPS = 1e-5
NC = 8          # cores
CH = 256        # tokens per core
VS = 6284       # padded vocab shard per core; 8*VS = 50272 >= V
KD = D // 128   # 4 k-tiles over D
MD = FF // 128  # 16 m-tiles over FF
NEG = -1.0e9


def build_program(reps=1, with_bias=True, layers=L, with_head=True, with_attn=True, head_mode='full'):
    nc = bacc.Bacc("TRN2", target_bir_lowering=False, debug=False, num_devices=NC)

    # ---- I/O ----
    x0 = nc.dram_tensor("x0", [CH, D], FP, kind="ExternalInput").ap()
    wq = nc.dram_tensor("wq", [L, D, D], FP, kind="ExternalInput").ap()
    wk = nc.dram_tensor("wk", [L, D, D], FP, kind="ExternalInput").ap()
    wv = nc.dram_tensor("wv", [L, D, D], FP, kind="ExternalInput").ap()
    wo = nc.dram_tensor("wo", [L, D, D], FP, kind="ExternalInput").ap()
    w1 = nc.dram_tensor("w1", [L, D, FF], FP, kind="ExternalInput").ap()
    w2 = nc.dram_tensor("w2", [L, FF, D], FP, kind="ExternalInput").ap()
    wlm = nc.dram_tensor("wlm", [D, VS], FP, kind="ExternalInput").ap()
    # bqk[p, l, 0/1, kd]: per-partition bias for qT/kT ([D] rearranged)
    bqk = nc.dram_tensor("bqk", [128, L, 2, KD], FP, kind="ExternalInput").ap()
    b1t = nc.dram_tensor("b1t", [128, L, MD], FP, kind="ExternalInput").ap()
    # bo2[l, 0]=bo_eff, [l, 1]=b2 (free-dim biases, broadcast via DMA)
    bo2 = nc.dram_tensor("bo2", [L, 2, D], FP, kind="ExternalInput").ap()
    # causal mask per core: [p, kchunk, ktile, 256 queries]
    msk = nc.dram_tensor("msk", [128, 4, 2, CH], FP, kind="ExternalInput").ap()
    logits = nc.dram_tensor("logits", [B * T, VS], FP, kind="ExternalOutput").ap()

    KV_K = 128 * KD * CH            # kT flat size per core
    KV_V = 128 * 2 * H * 65         # v_aug flat size per core
    KV = KV_K + KV_V
    XF = 128 * KD * CH              # xfT flat size

    from contextlib import ExitStack
    with ExitStack() as stk:
        tc = stk.enter_context(tile.TileContext(nc))
        ec = stk.enter_context
        consts = ec(tc.tile_pool(name="consts", bufs=1))
        xpool = ec(tc.tile_pool(name="xpool", bufs=1))
        hpool = ec(tc.tile_pool(name="hpool", bufs=2))
        t4 = ec(tc.tile_pool(name="t4", bufs=4))          # [128,KD,CH] transposed acts
        w4pool = ec(tc.tile_pool(name="w4", bufs=4))      # [128,KD,512] weights
        w1pool = ec(tc.tile_pool(name="w1k", bufs=4))     # [128,FF] W1 k-tiles
        w2pool = ec(tc.tile_pool(name="w2k", bufs=4))     # [128,512] W2 k-tiles
        kvall = ec(tc.tile_pool(name="kvall", bufs=1))
        vaugp = ec(tc.tile_pool(name="vaug", bufs=1))
        small = ec(tc.tile_pool(name="small", bufs=2))
        expp = ec(tc.tile_pool(name="exp", bufs=3))
        gtp = ec(tc.tile_pool(name="gt", bufs=2))
        lgp = ec(tc.tile_pool(name="lg", bufs=2))
        bcp = ec(tc.tile_pool(name="bcast", bufs=2))
        xftp = ec(tc.tile_pool(name="xft", bufs=2))
        mmp = ec(tc.tile_pool(name="mm", bufs=2, space="PSUM"))
        avp = ec(tc.tile_pool(name="avp", bufs=2, space="PSUM"))
        spp = ec(tc.tile_pool(name="sp", bufs=2, space="PSUM"))
        mop = ec(tc.tile_pool(name="mo", bufs=2, space="PSUM"))
        avacc = ec(tc.tile_pool(name="avacc", bufs=1))
        dram = ec(tc.tile_pool(name="dram", bufs=2, space="DRAM"))
        if True:
            ident = consts.tile([128, 128], FP)
            make_identity(nc, ident)
            epst = consts.tile([128, 1], FP)
            nc.vector.memset(epst, EPS)
            bqk_sb = consts.tile([128, L, 2, KD], FP)
            nc.sync.dma_start(bqk_sb[:], bqk)
            b1_sb = consts.tile([128, L, MD], FP)
            nc.sync.dma_start(b1_sb[:], b1t)
            msk_sb = consts.tile([128, 4, 2, CH], FP)
            nc.sync.dma_start(msk_sb[:], msk)



            def layernorm(src, tag):
                """src [128,2,D] -> fresh normalized tile [128,2,D] (no affine)."""
                out = hpool.tile([128, 2, D], FP, tag="h")
                for tt in range(2):
                    st = small.tile([128, 6], FP, tag="bnst")
                    nc.vector.bn_stats(st[:], src[:, tt, :])
                    mv = small.tile([128, 2], FP, tag="bnmv")
                    nc.vector.bn_aggr(mv[:], st[:])
                    nc.scalar.activation(mv[:, 1:2], mv[:, 1:2], AF.Sqrt,
                                         bias=epst[:, 0:1])
                    nc.vector.reciprocal(mv[:, 1:2], mv[:, 1:2])
                    nc.vector.tensor_scalar(
                        out=out[:, tt, :], in0=src[:, tt, :],
                        scalar1=mv[:, 0:1], scalar2=mv[:, 1:2],
                        op0=OP.subtract, op1=OP.mult)
                return out

            def transpose2(src, tag):
                """src [128,2,D] (tokens, dims) -> [128,KD,CH] (dims, tokens)."""
                out = t4.tile([128, KD, CH], FP, tag="t4")
                for d in range(KD):
                    for tt in range(2):
                        ps = mmp.tile([128, 512], FP, tag="mm")
                        nc.tensor.transpose(ps[:, :128], src[:, tt, ds(d * 128, 128)],
                                            ident[:])
                        nc.vector.tensor_copy(out[:, d, ds(tt * 128, 128)],
                                              ps[:, :128])
                return out

            def load_w4(src_ap, tag="w4"):
                w = w4pool.tile([128, KD, 512], FP, tag=tag)
                nc.sync.dma_start(w[:], src_ap.rearrange("(ko p) m -> p ko m", p=128))
                return w

            for rep in range(reps):
              xt = xpool.tile([128, 2, D], FP, tag="xt", name=f"xt{rep}")
              nc.sync.dma_start(xt[:], x0.rearrange("(tt p) d -> p tt d", p=128))
              for l in range(layers):
                # ---- LN1 + transpose ----
                h = layernorm(xt, "h")
                hT = transpose2(h, "t4")

                # ---- k/v first so the AllGather starts ASAP ----
                wk_sb = load_w4(wk[l])
                wv_sb = load_w4(wv[l])
                kT = t4.tile([128, KD, CH], FP, tag="t4")
                for d in range(KD):
                    ps = mmp.tile([128, 512], FP, tag="mm")
                    for k in range(KD):
                        nc.tensor.matmul(ps[:, :CH], wk_sb[:, k, ds(d * 128, 128)],
                                         hT[:, k, :], start=(k == 0),
                                         stop=(k == KD - 1))
                    nc.vector.tensor_scalar_add(kT[:, d, :], ps[:, :CH],
                                                bqk_sb[:, l, 1, d:d + 1])
                vaug = vaugp.tile([128, 2, H, 65], FP)
                nc.vector.memset(vaug[:, :, :, 64:65], 1.0)
                for tt in range(2):
                    ps = mmp.tile([128, 512], FP, tag="mm")
                    for k in range(KD):
                        nc.tensor.matmul(ps[:, :D], hT[:, k, ds(tt * 128, 128)],
                                         wv_sb[:, k, :], start=(k == 0),
                                         stop=(k == KD - 1))
                    for hh in range(H):
                        nc.vector.tensor_copy(vaug[:, tt, hh, 0:64],
                                              ps[:, ds(hh * 64, 64)])

                # ---- AllGather K,V across batch group ----
                kv_in = dram.tile([KV], FP, tag="kvin")
                nc.sync.dma_start(
                    kv_in[0:KV_K].rearrange("(p a b) -> p a b", p=128, a=KD), kT[:])
                nc.sync.dma_start(
                    kv_in[KV_K:KV].rearrange("(p a h e) -> p a h e", p=128, a=2, h=H),
                    vaug[:])
                kv_out = dram.tile([4, KV], FP, tag="kvout")
                nc.gpsimd.collective_compute(
                    "AllGather", OP.bypass,
                    replica_groups=[[0, 1, 2, 3], [4, 5, 6, 7]],
                    ins=[kv_in[:].opt()], outs=[kv_out[:].opt()])
                # ---- q projection overlaps the collective ----
                wq_sb = load_w4(wq[l])
                qT = t4.tile([128, KD, CH], FP, tag="t4")
                for d in range(KD):
                    ps = mmp.tile([128, 512], FP, tag="mm")
                    for k in range(KD):
                        nc.tensor.matmul(ps[:, :CH], wq_sb[:, k, ds(d * 128, 128)],
                                         hT[:, k, :], start=(k == 0),
                                         stop=(k == KD - 1))
                    nc.vector.tensor_scalar_add(qT[:, d, :], ps[:, :CH],
                                                bqk_sb[:, l, 0, d:d + 1])

                if not with_attn:
                    continue
                kTall = kvall.tile([128, KD, 4, CH], FP, tag="ktall")
                vall = kvall.tile([128, 4, 2, H, 65], FP, tag="vall")
                for c in range(4):
                    nc.sync.dma_start(
                        kTall[:, :, c, :],
                        kv_out[c, 0:KV_K].rearrange("(p a b) -> p a b", p=128, a=KD))
                    nc.sync.dma_start(
                        vall[:, c, :, :, :],
                        kv_out[c, KV_K:KV].rearrange("(p a h e) -> p a h e",
                                                     p=128, a=2, h=H))

                # ---- attention (transposed layout) ----
                # av accumulation lives in SBUF (PSUM is only 8 banks);
                # per-chunk AV psum tiles are transient.
                av_acc = avacc.tile([65, H, CH], FP, tag="avacc")
                for c in range(4):
                    for hh in range(H):
                        pb = (hh % 2) * 64
                        dt_ = hh // 2
                        avps = avp.tile([65, CH], FP, tag="av")
                        for kt in range(2):
                            sps = spp.tile([128, CH], FP, tag="sp")
                            nc.tensor.matmul(
                                sps[:],
                                kTall[pb:pb + 64, dt_, c, ds(kt * 128, 128)],
                                qT[pb:pb + 64, dt_, :],
                                start=True, stop=True)
                            ex = expp.tile([128, CH], FP, tag="exp")
                            nc.scalar.activation(ex[:], sps[:], AF.Exp)
                            nc.gpsimd.tensor_tensor(ex[:], ex[:],
                                                    msk_sb[:, c, kt, :], OP.mult)
                            nc.tensor.matmul(
                                avps[:], vall[:, c, kt, hh, :], ex[:],
                                start=(kt == 0), stop=(kt == 1))
                        if c == 0:
                            nc.vector.tensor_copy(av_acc[:, hh, :], avps[:])
                        else:
                            nc.vector.tensor_tensor(av_acc[:, hh, :],
                                                    av_acc[:, hh, :], avps[:],
                                                    OP.add)

                # ---- normalize by denominators (all in base-0 partition space;
                # DVE cannot shift partitions, DMA can) ----
                rdram = dram.tile([8, CH], FP, tag="rdram")
                nc.sync.dma_start(rdram[:], av_acc[64:65, :, :])
                rbc0 = avacc.tile([64, H, CH], FP, tag="rbc0")
                for hh in range(H):
                    nc.sync.dma_start(
                        rbc0[:, hh, :],
                        bass.AP(tensor=rdram.tensor, offset=rdram.offset + hh * CH,
                                ap=[[0, 64], [1, CH]]))
                nc.vector.reciprocal(rbc0[:], rbc0[:])
                attn_n = rbc0
                nc.vector.tensor_tensor(attn_n[:], av_acc[0:64, :, :], rbc0[:],
                                        OP.mult)
                attT = t4.tile([128, KD, CH], FP, tag="t4")
                for dt_ in range(KD):
                    for a in range(2):
                        nc.sync.dma_start(attT[a * 64:(a + 1) * 64, dt_, :],
                                          attn_n[:, 2 * dt_ + a, :])

                # ---- Wo + bias + residual ----
                wo_sb = load_w4(wo[l])
                if with_bias:
                    bo_b = bcp.tile([128, D], FP, tag="bc")
                    bo_src = bo2[l, 0]
                    nc.sync.dma_start(bo_b[:], bass.AP(
                        tensor=bo_src.tensor, offset=bo_src.offset,
                        ap=[[0, 128]] + list(bo_src.ap)))
                for tt in range(2):
                    ps = mmp.tile([128, 512], FP, tag="mm")
                    for k in range(KD):
                        nc.tensor.matmul(ps[:, :D], attT[:, k, ds(tt * 128, 128)],
                                         wo_sb[:, k, :], start=(k == 0),
                                         stop=(k == KD - 1))
                    if with_bias:
                        nc.vector.tensor_tensor(ps[:, :D], ps[:, :D], bo_b[:],
                                                OP.add)
                    nc.vector.tensor_tensor(xt[:, tt, :], xt[:, tt, :], ps[:, :D],
                                            OP.add)

                # ---- LN2 + transpose ----
                h2 = layernorm(xt, "h")
                h2T = transpose2(h2, "t4")

                # ---- MLP ----
                w1_sb = [w1pool.tile([128, FF], FP, tag="w1k", name=f"w1k{l}_{kk}") for kk in range(KD)]
                for k in range(KD):
                    nc.sync.dma_start(
                        w1_sb[k][:],
                        w1[l].rearrange("(ko p) f -> p ko f", p=128)[:, k, :])
                if with_bias:
                    b2_b = bcp.tile([128, D], FP, tag="bc")
                    b2_src = bo2[l, 1]
                    nc.sync.dma_start(b2_b[:], bass.AP(
                        tensor=b2_src.tensor, offset=b2_src.offset,
                        ap=[[0, 128]] + list(b2_src.ap)))
                x2ps = [mop.tile([128, D], FP, tag="mo", name=f"mo{l}_{kk}") for kk in range(2)]
                for m in range(MD):
                    gps = mmp.tile([128, 512], FP, tag="mm")
                    for k in range(KD):
                        nc.tensor.matmul(gps[:, :CH], w1_sb[k][:, ds(m * 128, 128)],
                                         h2T[:, k, :], start=(k == 0),
                                         stop=(k == KD - 1))
                    gt = gtp.tile([128, CH], FP, tag="gt")
                    nc.scalar.activation(gt[:], gps[:, :CH], AF.Gelu,
                                         bias=b1_sb[:, l, m:m + 1])
                    w2t = w2pool.tile([128, D], FP, tag="w2k")
                    nc.sync.dma_start(
                        w2t[:], w2[l].rearrange("(ko p) d -> p ko d", p=128)[:, m, :])
                    for tt in range(2):
                        nc.tensor.matmul(x2ps[tt][:], gt[:, ds(tt * 128, 128)],
                                         w2t[:], start=(m == 0), stop=(m == MD - 1))
                for tt in range(2):
                    if with_bias:
                        nc.vector.tensor_tensor(x2ps[tt][:], x2ps[tt][:], b2_b[:],
                                                OP.add)
                    nc.vector.tensor_tensor(xt[:, tt, :], xt[:, tt, :], x2ps[tt][:],
                                            OP.add)

              if not with_head:
                  continue
              # ---- final LN + AllGather hidden ----
              xf = layernorm(xt, "h")
              xfT = transpose2(xf, "t4")
              xf_in = dram.tile([XF], FP, tag="xfin")
              nc.sync.dma_start(
                  xf_in[:].rearrange("(p a b) -> p a b", p=128, a=KD), xfT[:])
              xf_out = dram.tile([NC, XF], FP, tag="xfout", addr_space="Shared")
              nc.gpsimd.collective_compute(
                  "AllGather", OP.bypass,
                  replica_groups=[list(range(NC))],
                  ins=[xf_in[:].opt()], outs=[xf_out[:].opt()])
              xfall = []
              for cg in range(2):
                  xa = xftp.tile([128, KD, 4, CH], FP, tag="xft")
                  for c in range(4):
                      nc.sync.dma_start(
                          xa[:, :, c, :],
                          xf_out[cg * 4 + c, :].rearrange("(p a b) -> p a b",
                                                          p=128, a=KD))
                  xfall.append(xa)

              # ---- LM head: vocab-sharded (ragged last tile) ----
              NT = (VS + 511) // 512
              hp_pools = [mmp, mop, spp]
              gi = 0
              for n in range(NT):
                  nsz = min(512, VS - n * 512)
                  wl = w4pool.tile([128, KD, 512], FP, tag="w4")
                  if head_mode != "nowlm" or n == 0:
                      nc.sync.dma_start(
                          wl[:, :, :nsz],
                          wlm.rearrange("(ko p) v -> p ko v", p=128)[:, :, ds(n * 512, nsz)])
                  for c in range(NC):
                      for mt in range(2):
                          pool_i, ptag = (
                              (mmp, "mm"), (mop, "mo"), (spp, "sp"))[gi % 3]
                          gi += 1
                          ps = pool_i.tile([128, 512], FP, tag=ptag,
                                           name=f"hps{n}_{c}_{mt}")
                          for k in range(KD):
                              nc.tensor.matmul(
                                  ps[:, :nsz],
                                  xfall[c // 4][:, k, c % 4, ds(mt * 128, 128)],
                                  wl[:, k, :nsz], start=(k == 0), stop=(k == KD - 1))
                          lg = w2pool.tile([128, 512], FP, tag="w2k",
                                           name=f"lg{n}_{c}_{mt}")
                          if head_mode != "nocopy":
                              nc.vector.tensor_copy(lg[:, :nsz], ps[:, :nsz])
                          if head_mode != "nodma":
                              nc.sync.dma_start(
                                  logits[ds(c * CH + mt * 128, 128),
                                         ds(n * 512, nsz)],
                                  lg[:, :nsz])
    nc.compile()
    return nc


_CACHE = {}


def _get_program(with_bias=True):
    key = ("nc", with_bias)
    if key not in _CACHE:
        _CACHE[key] = build_program(with_bias=with_bias)
    return _CACHE[key]


def _prep_inputs(inputs):
    f = lambda k: np.asarray(inputs[k], np.float32)
    idx = np.asarray(inputs["idx"]).astype(np.int64)
    tok_emb, pos_emb = f("tok_emb"), f("pos_emb")
    x0 = tok_emb[idx] + pos_emb[None, :T]          # [B, T, D]
    x0 = x0.reshape(NC, CH, D)

    ln1_g, ln1_b = f("ln1_g"), f("ln1_b")
    ln2_g, ln2_b = f("ln2_g"), f("ln2_b")
    Wq, bq = f("Wq"), f("bq")
    Wk, bk = f("Wk"), f("bk")
    Wv, bv = f("Wv"), f("bv")
    Wo, bo = f("Wo"), f("bo")
    W1, b1 = f("W1"), f("b1")
    W2, b2 = f("W2"), f("b2")
    lnf_g, lnf_b = f("lnf_g"), f("lnf_b")
    Wlm, blm = f("Wlm"), f("blm")

    sc = 1.0 / np.sqrt(HS)
    wqe = ln1_g[:, :, None] * Wq * sc
    bqe = (np.einsum("ld,ldm->lm", ln1_b, Wq) + bq) * sc
    wke = ln1_g[:, :, None] * Wk
    bke = np.einsum("ld,ldm->lm", ln1_b, Wk) + bk
    wve = ln1_g[:, :, None] * Wv
    bve = np.einsum("ld,ldm->lm", ln1_b, Wv) + bv
    boe = np.einsum("lm,lmd->ld", bve, Wo) + bo
    w1e = ln2_g[:, :, None] * W1
    b1e = np.einsum("ld,ldf->lf", ln2_b, W1) + b1
    wlme = lnf_g[:, None] * Wlm
    blme = lnf_b @ Wlm + blm

    bqk = np.stack([bqe, bke], axis=1)             # [L, 2, D]
    bqk = bqk.reshape(L, 2, KD, 128).transpose(3, 0, 1, 2).copy()
    b1t = b1e.reshape(L, MD, 128).transpose(2, 0, 1).copy()
    bo2 = np.stack([boe, b2], axis=1)              # [L, 2, D]

    wlmp = np.zeros((D, NC * VS), np.float32)
    wlmp[:, :V] = wlme

    # causal 0/1 masks per core (multiplied in after exp): [p, kc, kt, q]
    masks = []
    for core in range(NC):
        cc = core % 4
        qpos = cc * CH + np.arange(CH)
        m = np.empty((128, 4, 2, CH), np.float32)
        for kc in range(4):
            for kt in range(2):
                kpos = kc * CH + kt * 128 + np.arange(128)
                m[:, kc, kt, :] = (kpos[:, None] <= qpos[None, :]).astype(np.float32)
        masks.append(m)

    shared = dict(wq=np.ascontiguousarray(wqe), wk=np.ascontiguousarray(wke),
                  wv=np.ascontiguousarray(wve), wo=np.ascontiguousarray(Wo),
                  w1=np.ascontiguousarray(w1e), w2=np.ascontiguousarray(W2),
                  bqk=bqk, b1t=b1t, bo2=np.ascontiguousarray(bo2))
    in_maps = []
    for core in range(NC):
        m = dict(shared)
        m["x0"] = np.ascontiguousarray(x0[core])
        m["msk"] = masks[core]
        m["wlm"] = np.ascontiguousarray(wlmp[:, core * VS:(core + 1) * VS])
        in_maps.append(m)
    return in_maps, blme


def _run(inputs, trace=False):
    in_maps, blme = _prep_inputs(inputs)
    with_bias = bool(np.any(in_maps[0]["bo2"]))
    nc = _get_program(with_bias=with_bias)
    res = bass_utils.run_bass_kernel_spmd(nc, in_maps, core_ids=list(range(NC)),
                                          trace=trace)
    lg = np.concatenate([res.results[c]["logits"] for c in range(NC)], axis=1)
    out = lg[:, :V]
    if np.any(blme):
        out = out + blme[None, :]
    return out.reshape(B, T, V).astype(np.float32), res


def kernel(**inputs) -> np.ndarray:
    out, _ = _run(inputs, trace=False)
    return out



# revision 3
# speedup vs baseline: 2.2635x; 2.2635x over previous
"""Trainium2 Bass kernel for a 6-layer GPT (D=512, H=8, T=1024, B=2, V=50257).

Strategy (8 NeuronCores):
- Token-shard the transformer body: core c owns 256 tokens (cores 0-3 =
  batch 0 chunks 0-3, cores 4-7 = batch 1 chunks 0-3).
- All matmuls in bf16 (weights pre-cast host-side, activations cast on
  write); PSUM accumulation stays fp32; residual stream + LN stats fp32.
- Per layer: LN1 -> QKV (q/k in transposed [dim, tok] layout, v natural)
  -> AllGather K,V (bf16, Shared output) across the 4-core batch group
  -> causal attention in transposed layout; AV accumulates directly in
  PSUM across key chunks (2 heads in flight), softmax denominator fused
  as a ones-row of V; approx-reciprocal normalize -> Wo + residual ->
  LN2 -> MLP -> residual.
- Final LN -> AllGather hidden (bf16) across all 8 cores -> vocab-
  sharded LM head in bf16; logits stored bf16 and upcast host-side.
"""

import numpy as np
import ml_dtypes

import concourse.bass as bass
import concourse.tile as tile
from concourse import bacc, mybir
from concourse import bass_utils
from concourse.bass import ds, ts
from concourse.masks import make_identity

FP = mybir.dt.float32
BF = mybir.dt.bfloat16
AF = mybir.ActivationFunctionType
OP = mybir.AluOpType

V, D, T, L, H, HS, B = 50257, 512, 1024, 6, 8, 64, 2
FF = 4 * D
EPS = 1e-5
NC = 8          # cores
CH = 256        # tokens per core
VS = 6284       # padded vocab shard per core; 8*VS = 50272 >= V
KD = D // 128   # 4 k-tiles over D
MD = FF // 128  # 16 m-tiles over FF


def build_program(reps=1, with_bias=True, layers=L, with_head=True, with_attn=True, head_mode='full'):
    nc = bacc.Bacc("TRN2", target_bir_lowering=False, debug=False, num_devices=NC)

    # ---- I/O ----
    x0 = nc.dram_tensor("x0", [CH, D], FP, kind="ExternalInput").ap()
    wq = nc.dram_tensor("wq", [L, D, D], BF, kind="ExternalInput").ap()
    wk = nc.dram_tensor("wk", [L, D, D], BF, kind="ExternalInput").ap()
    wv = nc.dram_tensor("wv", [L, D, D], BF, kind="ExternalInput").ap()
    wo = nc.dram_tensor("wo", [L, D, D], BF, kind="ExternalInput").ap()
    w1 = nc.dram_tensor("w1", [L, D, FF], BF, kind="ExternalInput").ap()
    w2 = nc.dram_tensor("w2", [L, FF, D], BF, kind="ExternalInput").ap()
    wlm = nc.dram_tensor("wlm", [D, VS], BF, kind="ExternalInput").ap()
    # bqk[p, l, 0/1, kd]: per-partition bias for qT/kT ([D] rearranged)
    bqk = nc.dram_tensor("bqk", [128, L, 2, KD], FP, kind="ExternalInput").ap()
    b1t = nc.dram_tensor("b1t", [128, L, MD], FP, kind="ExternalInput").ap()
    # bo2[l, 0]=bo_eff, [l, 1]=b2 (free-dim biases, broadcast via DMA)
    bo2 = nc.dram_tensor("bo2", [L, 2, D], FP, kind="ExternalInput").ap()
    # causal mask per core: [p, kchunk, ktile, 256 queries]
    msk = nc.dram_tensor("msk", [128, 4, 2, CH], BF, kind="ExternalInput").ap()
    logits = nc.dram_tensor("logits", [B * T, VS], BF, kind="ExternalOutput").ap()

    KV_K = 128 * KD * CH            # kT flat size per core
    KV_V = 128 * 2 * H * 65         # v_aug flat size per core
    KV = KV_K + KV_V
    XF = 128 * KD * CH              # xfT flat size

    from contextlib import ExitStack
    with ExitStack() as stk:
        tc = stk.enter_context(tile.TileContext(nc))
        ec = stk.enter_context
        consts = ec(tc.tile_pool(name="consts", bufs=1))
        xpool = ec(tc.tile_pool(name="xpool", bufs=1))
        hpool = ec(tc.tile_pool(name="hpool", bufs=2))
        t4 = ec(tc.tile_pool(name="t4", bufs=4))          # [128,KD,CH] transposed acts
        w4pool = ec(tc.tile_pool(name="w4", bufs=4))      # [128,KD,512] weights
        w1pool = ec(tc.tile_pool(name="w1k", bufs=4))     # [128,FF] W1 k-tiles
        w2pool = ec(tc.tile_pool(name="w2k", bufs=4))     # [128,512] W2 k-tiles
        kvall = ec(tc.tile_pool(name="kvall", bufs=1))
        vaugp = ec(tc.tile_pool(name="vaug", bufs=1))
        small = ec(tc.tile_pool(name="small", bufs=2))
        expp = ec(tc.tile_pool(name="exp", bufs=3))
        gtp = ec(tc.tile_pool(name="gt", bufs=2))
        lgp = ec(tc.tile_pool(name="lg", bufs=2))
        bcp = ec(tc.tile_pool(name="bcast", bufs=2))
        xftp = ec(tc.tile_pool(name="xft", bufs=2))
        mmp = ec(tc.tile_pool(name="mm", bufs=2, space="PSUM"))
        avp = ec(tc.tile_pool(name="avp", bufs=2, space="PSUM"))
        spp = ec(tc.tile_pool(name="sp", bufs=2, space="PSUM"))
        mop = ec(tc.tile_pool(name="mo", bufs=2, space="PSUM"))
        avacc = ec(tc.tile_pool(name="avacc", bufs=1))
        dram = ec(tc.tile_pool(name="dram", bufs=2, space="DRAM"))
        if True:
            ident = consts.tile([128, 128], BF)
            make_identity(nc, ident)
            epst = consts.tile([128, 1], FP)
            nc.vector.memset(epst, EPS)
            bqk_sb = consts.tile([128, L, 2, KD], FP)
            nc.sync.dma_start(bqk_sb[:], bqk)
            b1_sb = consts.tile([128, L, MD], FP)
            nc.sync.dma_start(b1_sb[:], b1t)
            msk_sb = consts.tile([128, 4, 2, CH], BF)
            nc.sync.dma_start(msk_sb[:], msk)

            def layernorm(src, tag):
                """src [128,2,D] fp32 -> fresh normalized bf16 tile (no affine)."""
                out = hpool.tile([128, 2, D], BF, tag="h")
                for tt in range(2):
                    st = small.tile([128, 6], FP, tag="bnst")
                    nc.vector.bn_stats(st[:], src[:, tt, :])
                    mv = small.tile([128, 2], FP, tag="bnmv")
                    nc.vector.bn_aggr(mv[:], st[:])
                    nc.scalar.activation(mv[:, 1:2], mv[:, 1:2], AF.Sqrt,
                                         bias=epst[:, 0:1])
                    nc.vector.reciprocal(mv[:, 1:2], mv[:, 1:2])
                    nc.vector.tensor_scalar(
                        out=out[:, tt, :], in0=src[:, tt, :],
                        scalar1=mv[:, 0:1], scalar2=mv[:, 1:2],
                        op0=OP.subtract, op1=OP.mult)
                return out

            def transpose2(src, tag):
                """src [128,2,D] bf16 (tokens, dims) -> [128,KD,CH] (dims, tokens)."""
                out = t4.tile([128, KD, CH], BF, tag="t4")
                for d in range(KD):
                    for tt in range(2):
                        ps = mmp.tile([128, 128], BF, tag="mm")
                        nc.tensor.transpose(ps[:], src[:, tt, ds(d * 128, 128)],
                                            ident[:])
                        nc.vector.tensor_copy(out[:, d, ds(tt * 128, 128)],
                                              ps[:])
                return out

            def load_w4(src_ap, tag="w4"):
                w = w4pool.tile([128, KD, 512], BF, tag=tag)
                nc.sync.dma_start(w[:], src_ap.rearrange("(ko p) m -> p ko m", p=128))
                return w

            for rep in range(reps):
              xt = xpool.tile([128, 2, D], FP, tag="xt", name=f"xt{rep}")
              nc.sync.dma_start(xt[:], x0.rearrange("(tt p) d -> p tt d", p=128))
              for l in range(layers):
                # ---- LN1 + transpose ----
                h = layernorm(xt, "h")
                hT = transpose2(h, "t4")

                # ---- k/v first so the AllGather starts ASAP ----
                wk_sb = load_w4(wk[l])
                wv_sb = load_w4(wv[l])
                kT = t4.tile([128, KD, CH], BF, tag="t4")
                for d in range(KD):
                    ps = mmp.tile([128, 512], FP, tag="mm")
                    for k in range(KD):
                        nc.tensor.matmul(ps[:, :CH], wk_sb[:, k, ds(d * 128, 128)],
                                         hT[:, k, :], start=(k == 0),
                                         stop=(k == KD - 1))
                    nc.vector.tensor_scalar_add(kT[:, d, :], ps[:, :CH],
                                                bqk_sb[:, l, 1, d:d + 1])
                vaug = vaugp.tile([128, 2, H, 65], BF)
                nc.vector.memset(vaug[:, :, :, 64:65], 1.0)
                for tt in range(2):
                    ps = mmp.tile([128, 512], FP, tag="mm")
                    for k in range(KD):
                        nc.tensor.matmul(ps[:, :D], hT[:, k, ds(tt * 128, 128)],
                                         wv_sb[:, k, :], start=(k == 0),
                                         stop=(k == KD - 1))
                    for hh in range(H):
                        nc.vector.tensor_copy(vaug[:, tt, hh, 0:64],
                                              ps[:, ds(hh * 64, 64)])

                # ---- AllGather K,V across batch group ----
                kv_in = dram.tile([KV], BF, tag="kvin")
                nc.sync.dma_start(
                    kv_in[0:KV_K].rearrange("(p a b) -> p a b", p=128, a=KD), kT[:])
                nc.sync.dma_start(
                    kv_in[KV_K:KV].rearrange("(p a h e) -> p a h e", p=128, a=2, h=H),
                    vaug[:])
                kv_out = dram.tile([4, KV], BF, tag="kvout")
                nc.gpsimd.collective_compute(
                    "AllGather", OP.bypass,
                    replica_groups=[[0, 1, 2, 3], [4, 5, 6, 7]],
                    ins=[kv_in[:].opt()], outs=[kv_out[:].opt()])
                # ---- q projection overlaps the collective ----
                wq_sb = load_w4(wq[l])
                qT = t4.tile([128, KD, CH], BF, tag="t4")
                for d in range(KD):
                    ps = mmp.tile([128, 512], FP, tag="mm")
                    for k in range(KD):
                        nc.tensor.matmul(ps[:, :CH], wq_sb[:, k, ds(d * 128, 128)],
                                         hT[:, k, :], start=(k == 0),
                                         stop=(k == KD - 1))
                    nc.vector.tensor_scalar_add(qT[:, d, :], ps[:, :CH],
                                                bqk_sb[:, l, 0, d:d + 1])

                if not with_attn:
                    continue
                kTall = kvall.tile([128, KD, 4, CH], BF, tag="ktall")
                vall = kvall.tile([128, 4, 2, H, 65], BF, tag="vall")
                for c in range(4):
                    nc.sync.dma_start(
                        kTall[:, :, c, :],
                        kv_out[c, 0:KV_K].rearrange("(p a b) -> p a b", p=128, a=KD))
                    nc.sync.dma_start(
                        vall[:, c, :, :, :],
                        kv_out[c, KV_K:KV].rearrange("(p a h e) -> p a h e",
                                                     p=128, a=2, h=H))

                # ---- attention (transposed layout) ----
                # AV accumulates in PSUM across all key chunks; 2 heads in
                # flight per pass (2 PSUM banks), 4 passes over the heads.
                denrow = avacc.tile([65, H, CH], FP, tag="denrow")
                attn_nb = avacc.tile([64, H, CH], BF, tag="attnb")
                for pas in range(4):
                    hds = (2 * pas, 2 * pas + 1)
                    av_ps = {}
                    for hh in hds:
                        av_ps[hh] = avp.tile([65, CH], FP, tag="av",
                                             name=f"av{l}_{hh}")
                    for c in range(4):
                        for hh in hds:
                            pb = (hh % 2) * 64
                            dt_ = hh // 2
                            for kt in range(2):
                                sps = spp.tile([128, CH], FP, tag="sp")
                                nc.tensor.matmul(
                                    sps[:],
                                    kTall[pb:pb + 64, dt_, c, ds(kt * 128, 128)],
                                    qT[pb:pb + 64, dt_, :],
                                    start=True, stop=True)
                                ex = expp.tile([128, CH], BF, tag="exp")
                                nc.scalar.activation(ex[:], sps[:], AF.Exp)
                                nc.vector.tensor_tensor(ex[:], ex[:],
                                                        msk_sb[:, c, kt, :], OP.mult)
                                nc.tensor.matmul(
                                    av_ps[hh][:], vall[:, c, kt, hh, :], ex[:],
                                    start=(c == 0 and kt == 0),
                                    stop=(c == 3 and kt == 1))
                    # ---- normalize these 2 heads (overlaps next pass) ----
                    for hh in hds:
                        nc.vector.tensor_copy(denrow[64:65, hh, :],
                                              av_ps[hh][64:65, :])
                    rdram = dram.tile([2, CH], FP, tag="rdram",
                                      name=f"rd{l}_{pas}")
                    nc.sync.dma_start(rdram[:], denrow[64:65, ds(2 * pas, 2), :])
                    rbc = avacc.tile([64, 2, CH], FP, tag="rbc")
                    for i in range(2):
                        nc.sync.dma_start(
                            rbc[:, i, :],
                            bass.AP(tensor=rdram.tensor,
                                    offset=rdram.offset + i * CH,
                                    ap=[[0, 64], [1, CH]]))
                    nc.vector.reciprocal_approx_fast(rbc[:], rbc[:])
                    for i, hh in enumerate(hds):
                        nc.vector.tensor_tensor(attn_nb[:, hh, :],
                                                av_ps[hh][0:64, :], rbc[:, i, :],
                                                OP.mult)
                attT = t4.tile([128, KD, CH], BF, tag="t4")
                for dt_ in range(KD):
                    for a in range(2):
                        nc.sync.dma_start(attT[a * 64:(a + 1) * 64, dt_, :],
                                          attn_nb[:, 2 * dt_ + a, :])

                # ---- Wo + bias + residual ----
                wo_sb = load_w4(wo[l])
                if with_bias:
                    bo_b = bcp.tile([128, D], FP, tag="bc")
                    bo_src = bo2[l, 0]
                    nc.sync.dma_start(bo_b[:], bass.AP(
                        tensor=bo_src.tensor, offset=bo_src.offset,
                        ap=[[0, 128]] + list(bo_src.ap)))
                for tt in range(2):
                    ps = mmp.tile([128, 512], FP, tag="mm")
                    for k in range(KD):
                        nc.tensor.matmul(ps[:, :D], attT[:, k, ds(tt * 128, 128)],
                                         wo_sb[:, k, :], start=(k == 0),
                                         stop=(k == KD - 1))
                    if with_bias:
                        nc.vector.tensor_tensor(ps[:, :D], ps[:, :D], bo_b[:],
                                                OP.add)
                    nc.vector.tensor_tensor(xt[:, tt, :], xt[:, tt, :], ps[:, :D],
                                            OP.add)

                # ---- LN2 + transpose ----
                h2 = layernorm(xt, "h")
                h2T = transpose2(h2, "t4")

                # ---- MLP ----
                w1_sb = [w1pool.tile([128, FF], BF, tag="w1k", name=f"w1k{l}_{kk}") for kk in range(KD)]
                for k in range(KD):
                    nc.sync.dma_start(
                        w1_sb[k][:],
                        w1[l].rearrange("(ko p) f -> p ko f", p=128)[:, k, :])
                if with_bias:
                    b2_b = bcp.tile([128, D], FP, tag="bc")
                    b2_src = bo2[l, 1]
                    nc.sync.dma_start(b2_b[:], bass.AP(
                        tensor=b2_src.tensor, offset=b2_src.offset,
                        ap=[[0, 128]] + list(b2_src.ap)))
                x2ps = [mop.tile([128, D], FP, tag="mo", name=f"mo{l}_{kk}") for kk in range(2)]
                for m in range(MD):
                    gps = mmp.tile([128, 512], FP, tag="mm")
                    for k in range(KD):
                        nc.tensor.matmul(gps[:, :CH], w1_sb[k][:, ds(m * 128, 128)],
                                         h2T[:, k, :], start=(k == 0),
                                         stop=(k == KD - 1))
                    gt = gtp.tile([128, CH], BF, tag="gt")
                    nc.scalar.activation(gt[:], gps[:, :CH], AF.Gelu,
                                         bias=b1_sb[:, l, m:m + 1])
                    w2t = w2pool.tile([128, D], BF, tag="w2k")
                    nc.sync.dma_start(
                        w2t[:], w2[l].rearrange("(ko p) d -> p ko d", p=128)[:, m, :])
                    for tt in range(2):
                        nc.tensor.matmul(x2ps[tt][:], gt[:, ds(tt * 128, 128)],
                                         w2t[:], start=(m == 0), stop=(m == MD - 1))
                for tt in range(2):
                    if with_bias:
                        nc.vector.tensor_tensor(x2ps[tt][:], x2ps[tt][:], b2_b[:],
                                                OP.add)
                    nc.vector.tensor_tensor(xt[:, tt, :], xt[:, tt, :], x2ps[tt][:],
                                            OP.add)

              if not with_head:
                  continue
              # ---- final LN + AllGather hidden ----
              xf = layernorm(xt, "h")
              xfT = transpose2(xf, "t4")
              xf_in = dram.tile([XF], BF, tag="xfin")
              nc.sync.dma_start(
                  xf_in[:].rearrange("(p a b) -> p a b", p=128, a=KD), xfT[:])
              xf_out = dram.tile([NC, XF], BF, tag="xfout", addr_space="Shared")
              nc.gpsimd.collective_compute(
                  "AllGather", OP.bypass,
                  replica_groups=[list(range(NC))],
                  ins=[xf_in[:].opt()], outs=[xf_out[:].opt()])
              xfall = []
              for cg in range(2):
                  xa = xftp.tile([128, KD, 4, CH], BF, tag="xft")
                  for c in range(4):
                      nc.sync.dma_start(
                          xa[:, :, c, :],
                          xf_out[cg * 4 + c, :].rearrange("(p a b) -> p a b",
                                                          p=128, a=KD))
                  xfall.append(xa)

              # ---- LM head: vocab-sharded (ragged last tile) ----
              NT = (VS + 511) // 512
              gi = 0
              for n in range(NT):
                  nsz = min(512, VS - n * 512)
                  wl = w4pool.tile([128, KD, 512], BF, tag="w4")
                  if head_mode != "nowlm" or n == 0:
                      nc.sync.dma_start(
                          wl[:, :, :nsz],
                          wlm.rearrange("(ko p) v -> p ko v", p=128)[:, :, ds(n * 512, nsz)])
                  for c in range(NC):
                      for mt in range(2):
                          pool_i, ptag = (
                              (mmp, "mm"), (mop, "mo"), (spp, "sp"))[gi % 3]
                          ps = pool_i.tile([128, 512], FP, tag=ptag,
                                           name=f"hps{n}_{c}_{mt}")
                          for k in range(KD):
                              nc.tensor.matmul(
                                  ps[:, :nsz],
                                  xfall[c // 4][:, k, c % 4, ds(mt * 128, 128)],
                                  wl[:, k, :nsz], start=(k == 0), stop=(k == KD - 1))
                          lg = lgp.tile([128, 512], BF, tag="lg",
                                        name=f"lg{n}_{c}_{mt}")
                          if head_mode != "nocopy":
                              if gi % 2 == 0:
                                  nc.scalar.copy(lg[:, :nsz], ps[:, :nsz])
                              else:
                                  nc.vector.tensor_copy(lg[:, :nsz], ps[:, :nsz])
                          gi += 1
                          if head_mode != "nodma":
                              nc.sync.dma_start(
                                  logits[ds(c * CH + mt * 128, 128),
                                         ds(n * 512, nsz)],
                                  lg[:, :nsz])
    nc.compile()
    return nc


_CACHE = {}


def _get_program(with_bias=True):
    key = ("nc", with_bias)
    if key not in _CACHE:
        _CACHE[key] = build_program(with_bias=with_bias)
    return _CACHE[key]


def _prep_inputs(inputs):
    f = lambda k: np.asarray(inputs[k], np.float32)
    bf = ml_dtypes.bfloat16
    idx = np.asarray(inputs["idx"]).astype(np.int64)
    tok_emb, pos_emb = f("tok_emb"), f("pos_emb")
    x0 = tok_emb[idx] + pos_emb[None, :T]          # [B, T, D]
    x0 = x0.reshape(NC, CH, D)

    ln1_g, ln1_b = f("ln1_g"), f("ln1_b")
    ln2_g, ln2_b = f("ln2_g"), f("ln2_b")
    Wq, bq = f("Wq"), f("bq")
    Wk, bk = f("Wk"), f("bk")
    Wv, bv = f("Wv"), f("bv")
    Wo, bo = f("Wo"), f("bo")
    W1, b1 = f("W1"), f("b1")
    W2, b2 = f("W2"), f("b2")
    lnf_g, lnf_b = f("lnf_g"), f("lnf_b")
    Wlm, blm = f("Wlm"), f("blm")

    sc = 1.0 / np.sqrt(HS)
    wqe = ln1_g[:, :, None] * Wq * sc
    bqe = (np.einsum("ld,ldm->lm", ln1_b, Wq) + bq) * sc
    wke = ln1_g[:, :, None] * Wk
    bke = np.einsum("ld,ldm->lm", ln1_b, Wk) + bk
    wve = ln1_g[:, :, None] * Wv
    bve = np.einsum("ld,ldm->lm", ln1_b, Wv) + bv
    boe = np.einsum("lm,lmd->ld", bve, Wo) + bo
    w1e = ln2_g[:, :, None] * W1
    b1e = np.einsum("ld,ldf->lf", ln2_b, W1) + b1
    wlme = lnf_g[:, None] * Wlm
    blme = lnf_b @ Wlm + blm

    bqk = np.stack([bqe, bke], axis=1)             # [L, 2, D]
    bqk = bqk.reshape(L, 2, KD, 128).transpose(3, 0, 1, 2).copy()
    b1t = b1e.reshape(L, MD, 128).transpose(2, 0, 1).copy()
    bo2 = np.stack([boe, b2], axis=1)              # [L, 2, D]

    wlmp = np.zeros((D, NC * VS), np.float32)
    wlmp[:, :V] = wlme

    # causal 0/1 masks per core (multiplied in after exp): [p, kc, kt, q]
    masks = []
    for core in range(NC):
        cc = core % 4
        qpos = cc * CH + np.arange(CH)
        m = np.empty((128, 4, 2, CH), np.float32)
        for kc in range(4):
            for kt in range(2):
                kpos = kc * CH + kt * 128 + np.arange(128)
                m[:, kc, kt, :] = (kpos[:, None] <= qpos[None, :]).astype(np.float32)
        masks.append(m.astype(bf))

    shared = dict(wq=np.ascontiguousarray(wqe.astype(bf)),
                  wk=np.ascontiguousarray(wke.astype(bf)),
                  wv=np.ascontiguousarray(wve.astype(bf)),
                  wo=np.ascontiguousarray(Wo.astype(bf)),
                  w1=np.ascontiguousarray(w1e.astype(bf)),
                  w2=np.ascontiguousarray(W2.astype(bf)),
                  bqk=bqk, b1t=b1t, bo2=np.ascontiguousarray(bo2))
    in_maps = []
    for core in range(NC):
        m = dict(shared)
        m["x0"] = np.ascontiguousarray(x0[core])
        m["msk"] = masks[core]
        m["wlm"] = np.ascontiguousarray(
            wlmp[:, core * VS:(core + 1) * VS].astype(bf))
        in_maps.append(m)
    return in_maps, blme


def _run(inputs, trace=False):
    in_maps, blme = _prep_inputs(inputs)
    with_bias = bool(np.any(in_maps[0]["bo2"]))
    nc = _get_program(with_bias=with_bias)
    res = bass_utils.run_bass_kernel_spmd(nc, in_maps, core_ids=list(range(NC)),
                                          trace=trace)
    lg = np.concatenate([np.asarray(res.results[c]["logits"], np.float32)
                         for c in range(NC)], axis=1)
    out = lg[:, :V]
    if np.any(blme):
        out = out + blme[None, :]
    return out.reshape(B, T, V).astype(np.float32), res


def kernel(**inputs) -> np.ndarray:
    out, _ = _run(inputs, trace=False)
    return out


# revision 8
# speedup vs baseline: 2.4695x; 1.0910x over previous
"""Trainium2 Bass kernel for a 6-layer GPT (D=512, H=8, T=1024, B=2, V=50257).

Strategy (8 NeuronCores):
- Token-shard the transformer body: core c owns 256 tokens (cores 0-3 =
  batch 0 chunks 0-3, cores 4-7 = batch 1 chunks 0-3).
- All matmuls in bf16 (weights pre-cast host-side, activations cast on
  write); PSUM accumulation stays fp32; residual stream + LN stats fp32.
- Per layer: LN1 -> QKV (q/k in transposed [dim, tok] layout, v natural)
  -> AllGather K,V (bf16, Shared output) across the 4-core batch group
  -> causal attention in transposed layout; AV accumulates directly in
  PSUM across key chunks (2 heads in flight), softmax denominator fused
  as a ones-row of V; approx-reciprocal normalize -> Wo + residual ->
  LN2 -> MLP -> residual.
- Final LN -> AllGather hidden (bf16) across all 8 cores -> vocab-
  sharded LM head in bf16; logits stored bf16 and upcast host-side.
"""

import numpy as np
import ml_dtypes

import concourse.bass as bass
import concourse.tile as tile
from concourse import bacc, mybir
from concourse import bass_utils
from concourse.bass import ds, ts
from concourse.masks import make_identity

FP = mybir.dt.float32
BF = mybir.dt.bfloat16
AF = mybir.ActivationFunctionType
OP = mybir.AluOpType

V, D, T, L, H, HS, B = 50257, 512, 1024, 6, 8, 64, 2
FF = 4 * D
EPS = 1e-5
NC = 8          # cores
CH = 256        # tokens per core
VS = 6400       # padded vocab shard per core; 8*VS = 51200 >= V
KD = D // 128   # 4 k-tiles over D
MD = FF // 128  # 16 m-tiles over FF


def build_program(reps=1, with_bias=True, layers=L, with_head=True, with_attn=True, head_mode='full'):
    nc = bacc.Bacc("TRN2", target_bir_lowering=False, debug=False, num_devices=NC)

    # ---- I/O ----
    x0 = nc.dram_tensor("x0", [CH, D], FP, kind="ExternalInput").ap()
    wq = nc.dram_tensor("wq", [L, D, D], BF, kind="ExternalInput").ap()
    wk = nc.dram_tensor("wk", [L, D, D], BF, kind="ExternalInput").ap()
    wv = nc.dram_tensor("wv", [L, D, D], BF, kind="ExternalInput").ap()
    wo = nc.dram_tensor("wo", [L, D, D], BF, kind="ExternalInput").ap()
    w1 = nc.dram_tensor("w1", [L, D, FF], BF, kind="ExternalInput").ap()
    w2 = nc.dram_tensor("w2", [L, FF, D], BF, kind="ExternalInput").ap()
    wlm = nc.dram_tensor("wlm", [D, VS], BF, kind="ExternalInput").ap()
    # bqk[p, l, 0/1, kd]: per-partition bias for qT/kT ([D] rearranged)
    bqk = nc.dram_tensor("bqk", [128, L, 2, KD], FP, kind="ExternalInput").ap()
    b1t = nc.dram_tensor("b1t", [128, L, MD], FP, kind="ExternalInput").ap()
    # bo2[l, 0]=bo_eff, [l, 1]=b2 (free-dim biases, broadcast via DMA)
    bo2 = nc.dram_tensor("bo2", [L, 2, D], FP, kind="ExternalInput").ap()
    # causal mask per core: [p, kchunk, ktile, 256 queries]
    msk = nc.dram_tensor("msk", [128, 4, 2, CH], BF, kind="ExternalInput").ap()
    # transposed logits: rows = vocab shard, cols = tokens (contiguous DMA)
    logits = nc.dram_tensor("logits", [VS, B * T], BF, kind="ExternalOutput").ap()

    KV_K = 128 * KD * CH            # kT flat size per core
    KV_V = 128 * 2 * H * 65         # v_aug flat size per core
    KV = KV_K + KV_V
    XF = 128 * KD * CH              # xfT flat size

    from contextlib import ExitStack
    with ExitStack() as stk:
        tc = stk.enter_context(tile.TileContext(nc))
        ec = stk.enter_context
        consts = ec(tc.tile_pool(name="consts", bufs=1))
        xpool = ec(tc.tile_pool(name="xpool", bufs=1))
        hpool = ec(tc.tile_pool(name="hpool", bufs=2))
        t4 = ec(tc.tile_pool(name="t4", bufs=4))          # [128,KD,CH] transposed acts
        w4pool = ec(tc.tile_pool(name="w4", bufs=4))      # [128,KD,512] weights
        w1pool = ec(tc.tile_pool(name="w1k", bufs=4))     # [128,FF] W1 k-tiles
        w2pool = ec(tc.tile_pool(name="w2k", bufs=4))     # [128,512] W2 k-tiles
        kvall = ec(tc.tile_pool(name="kvall", bufs=1))
        vaugp = ec(tc.tile_pool(name="vaug", bufs=1))
        small = ec(tc.tile_pool(name="small", bufs=2))
        expp = ec(tc.tile_pool(name="exp", bufs=3))
        gtp = ec(tc.tile_pool(name="gt", bufs=2))
        lgp = ec(tc.tile_pool(name="lg", bufs=2))
        bcp = ec(tc.tile_pool(name="bcast", bufs=2))
        xftp = ec(tc.tile_pool(name="xft", bufs=2))
        mmp = ec(tc.tile_pool(name="mm", bufs=2, space="PSUM"))
        avp = ec(tc.tile_pool(name="avp", bufs=2, space="PSUM"))
        spp = ec(tc.tile_pool(name="sp", bufs=2, space="PSUM"))
        mop = ec(tc.tile_pool(name="mo", bufs=2, space="PSUM"))
        avacc = ec(tc.tile_pool(name="avacc", bufs=1))
        dram = ec(tc.tile_pool(name="dram", bufs=2, space="DRAM"))
        if True:
            ident = consts.tile([128, 128], BF)
            make_identity(nc, ident)
            epst = consts.tile([128, 1], FP)
            nc.vector.memset(epst, EPS)
            bqk_sb = consts.tile([128, L, 2, KD], FP)
            nc.sync.dma_start(bqk_sb[:], bqk)
            b1_sb = consts.tile([128, L, MD], FP)
            nc.sync.dma_start(b1_sb[:], b1t)
            msk_sb = consts.tile([128, 4, 2, CH], BF)
            nc.sync.dma_start(msk_sb[:], msk)

            def layernorm(src, tag):
                """src [128,2,D] fp32 -> fresh normalized bf16 tile (no affine)."""
                out = hpool.tile([128, 2, D], BF, tag="h")
                for tt in range(2):
                    st = small.tile([128, 6], FP, tag="bnst")
                    nc.vector.bn_stats(st[:], src[:, tt, :])
                    mv = small.tile([128, 2], FP, tag="bnmv")
                    nc.vector.bn_aggr(mv[:], st[:])
                    nc.scalar.activation(mv[:, 1:2], mv[:, 1:2], AF.Sqrt,
                                         bias=epst[:, 0:1])
                    nc.vector.reciprocal(mv[:, 1:2], mv[:, 1:2])
                    nc.vector.tensor_scalar(
                        out=out[:, tt, :], in0=src[:, tt, :],
                        scalar1=mv[:, 0:1], scalar2=mv[:, 1:2],
                        op0=OP.subtract, op1=OP.mult)
                return out

            def transpose2(src, tag):
                """src [128,2,D] bf16 (tokens, dims) -> [128,KD,CH] (dims, tokens)."""
                out = t4.tile([128, KD, CH], BF, tag="t4")
                for d in range(KD):
                    for tt in range(2):
                        ps = mmp.tile([128, 128], BF, tag="mm")
                        nc.tensor.transpose(ps[:], src[:, tt, ds(d * 128, 128)],
                                            ident[:])
                        nc.vector.tensor_copy(out[:, d, ds(tt * 128, 128)],
                                              ps[:])
                return out

            def load_w4(src_ap, tag="w4"):
                w = w4pool.tile([128, KD, 512], BF, tag=tag)
                nc.sync.dma_start(w[:], src_ap.rearrange("(ko p) m -> p ko m", p=128))
                return w

            for rep in range(reps):
              xt = xpool.tile([128, 2, D], FP, tag="xt", name=f"xt{rep}")
              nc.sync.dma_start(xt[:], x0.rearrange("(tt p) d -> p tt d", p=128))
              for l in range(layers):
                # ---- LN1 + transpose ----
                h = layernorm(xt, "h")
                hT = transpose2(h, "t4")

                # ---- k/v first so the AllGather starts ASAP ----
                wk_sb = load_w4(wk[l])
                wv_sb = load_w4(wv[l])
                kT = t4.tile([128, KD, CH], BF, tag="t4")
                for d in range(KD):
                    ps = mmp.tile([128, 512], FP, tag="mm")
                    for k in range(KD):
                        nc.tensor.matmul(ps[:, :CH], wk_sb[:, k, ds(d * 128, 128)],
                                         hT[:, k, :], start=(k == 0),
                                         stop=(k == KD - 1))
                    nc.vector.tensor_scalar_add(kT[:, d, :], ps[:, :CH],
                                                bqk_sb[:, l, 1, d:d + 1])
                vaug = vaugp.tile([128, 2, H, 65], BF)
                nc.vector.memset(vaug[:, :, :, 64:65], 1.0)
                for tt in range(2):
                    ps = mmp.tile([128, 512], FP, tag="mm")
                    for k in range(KD):
                        nc.tensor.matmul(ps[:, :D], hT[:, k, ds(tt * 128, 128)],
                                         wv_sb[:, k, :], start=(k == 0),
                                         stop=(k == KD - 1))
                    for hh in range(H):
                        nc.vector.tensor_copy(vaug[:, tt, hh, 0:64],
                                              ps[:, ds(hh * 64, 64)])

                # ---- AllGather K,V across batch group ----
                kv_in = dram.tile([KV], BF, tag="kvin")
                nc.sync.dma_start(
                    kv_in[0:KV_K].rearrange("(p a b) -> p a b", p=128, a=KD), kT[:])
                nc.sync.dma_start(
                    kv_in[KV_K:KV].rearrange("(p a h e) -> p a h e", p=128, a=2, h=H),
                    vaug[:])
                kv_out = dram.tile([4, KV], BF, tag="kvout")
                nc.gpsimd.collective_compute(
                    "AllGather", OP.bypass,
                    replica_groups=[[0, 1, 2, 3], [4, 5, 6, 7]],
                    ins=[kv_in[:].opt()], outs=[kv_out[:].opt()])
                # ---- q projection overlaps the collective ----
                wq_sb = load_w4(wq[l])
                qT = t4.tile([128, KD, CH], BF, tag="t4")
                for d in range(KD):
                    ps = mmp.tile([128, 512], FP, tag="mm")
                    for k in range(KD):
                        nc.tensor.matmul(ps[:, :CH], wq_sb[:, k, ds(d * 128, 128)],
                                         hT[:, k, :], start=(k == 0),
                                         stop=(k == KD - 1))
                    nc.vector.tensor_scalar_add(qT[:, d, :], ps[:, :CH],
                                                bqk_sb[:, l, 0, d:d + 1])

                if not with_attn:
                    continue
                kTall = kvall.tile([128, KD, 4, CH], BF, tag="ktall")
                vall = kvall.tile([128, 4, 2, H, 65], BF, tag="vall")
                for c in range(4):
                    nc.sync.dma_start(
                        kTall[:, :, c, :],
                        kv_out[c, 0:KV_K].rearrange("(p a b) -> p a b", p=128, a=KD))
                    nc.sync.dma_start(
                        vall[:, c, :, :, :],
                        kv_out[c, KV_K:KV].rearrange("(p a h e) -> p a h e",
                                                     p=128, a=2, h=H))

                # ---- attention (transposed layout) ----
                # AV accumulates in PSUM across all key chunks; 2 heads in
                # flight per pass (2 PSUM banks), 4 passes over the heads.
                denrow = avacc.tile([65, H, CH], FP, tag="denrow")
                attn_nb = avacc.tile([64, H, CH], BF, tag="attnb")
                for pas in range(4):
                    hds = (2 * pas, 2 * pas + 1)
                    av_ps = {}
                    for hh in hds:
                        av_ps[hh] = avp.tile([65, CH], FP, tag="av",
                                             name=f"av{l}_{hh}")
                    for c in range(4):
                        for hh in hds:
                            pb = (hh % 2) * 64
                            dt_ = hh // 2
                            for kt in range(2):
                                sps = spp.tile([128, CH], FP, tag="sp")
                                nc.tensor.matmul(
                                    sps[:],
                                    kTall[pb:pb + 64, dt_, c, ds(kt * 128, 128)],
                                    qT[pb:pb + 64, dt_, :],
                                    start=True, stop=True)
                                ex = expp.tile([128, CH], BF, tag="exp")
                                nc.scalar.activation(ex[:], sps[:], AF.Exp)
                                nc.vector.tensor_tensor(ex[:], ex[:],
                                                        msk_sb[:, c, kt, :], OP.mult)
                                nc.tensor.matmul(
                                    av_ps[hh][:], vall[:, c, kt, hh, :], ex[:],
                                    start=(c == 0 and kt == 0),
                                    stop=(c == 3 and kt == 1))
                    # ---- normalize these 2 heads (overlaps next pass) ----
                    for hh in hds:
                        nc.vector.tensor_copy(denrow[64:65, hh, :],
                                              av_ps[hh][64:65, :])
                    rdram = dram.tile([2, CH], FP, tag="rdram",
                                      name=f"rd{l}_{pas}")
                    nc.sync.dma_start(rdram[:], denrow[64:65, ds(2 * pas, 2), :])
                    rbc = avacc.tile([64, 2, CH], FP, tag="rbc")
                    for i in range(2):
                        nc.sync.dma_start(
                            rbc[:, i, :],
                            bass.AP(tensor=rdram.tensor,
                                    offset=rdram.offset + i * CH,
                                    ap=[[0, 64], [1, CH]]))
                    nc.vector.reciprocal_approx_fast(rbc[:], rbc[:])
                    for i, hh in enumerate(hds):
                        nc.vector.tensor_tensor(attn_nb[:, hh, :],
                                                av_ps[hh][0:64, :], rbc[:, i, :],
                                                OP.mult)
                attT = t4.tile([128, KD, CH], BF, tag="t4")
                for dt_ in range(KD):
                    for a in range(2):
                        nc.sync.dma_start(attT[a * 64:(a + 1) * 64, dt_, :],
                                          attn_nb[:, 2 * dt_ + a, :])

                # ---- Wo + bias + residual ----
                wo_sb = load_w4(wo[l])
                if with_bias:
                    bo_b = bcp.tile([128, D], FP, tag="bc")
                    bo_src = bo2[l, 0]
                    nc.sync.dma_start(bo_b[:], bass.AP(
                        tensor=bo_src.tensor, offset=bo_src.offset,
                        ap=[[0, 128]] + list(bo_src.ap)))
                for tt in range(2):
                    ps = mmp.tile([128, 512], FP, tag="mm")
                    for k in range(KD):
                        nc.tensor.matmul(ps[:, :D], attT[:, k, ds(tt * 128, 128)],
                                         wo_sb[:, k, :], start=(k == 0),
                                         stop=(k == KD - 1))
                    if with_bias:
                        nc.vector.tensor_tensor(ps[:, :D], ps[:, :D], bo_b[:],
                                                OP.add)
                    nc.vector.tensor_tensor(xt[:, tt, :], xt[:, tt, :], ps[:, :D],
                                            OP.add)

                # ---- LN2 + transpose ----
                h2 = layernorm(xt, "h")
                h2T = transpose2(h2, "t4")

                # ---- MLP ----
                w1_sb = [w1pool.tile([128, FF], BF, tag="w1k", name=f"w1k{l}_{kk}") for kk in range(KD)]
                for k in range(KD):
                    nc.sync.dma_start(
                        w1_sb[k][:],
                        w1[l].rearrange("(ko p) f -> p ko f", p=128)[:, k, :])
                if with_bias:
                    b2_b = bcp.tile([128, D], FP, tag="bc")
                    b2_src = bo2[l, 1]
                    nc.sync.dma_start(b2_b[:], bass.AP(
                        tensor=b2_src.tensor, offset=b2_src.offset,
                        ap=[[0, 128]] + list(b2_src.ap)))
                x2ps = [mop.tile([128, D], FP, tag="mo", name=f"mo{l}_{kk}") for kk in range(2)]
                for m in range(MD):
                    gps = mmp.tile([128, 512], FP, tag="mm")
                    for k in range(KD):
                        nc.tensor.matmul(gps[:, :CH], w1_sb[k][:, ds(m * 128, 128)],
                                         h2T[:, k, :], start=(k == 0),
                                         stop=(k == KD - 1))
                    gt = gtp.tile([128, CH], BF, tag="gt")
                    nc.scalar.activation(gt[:], gps[:, :CH], AF.Gelu,
                                         bias=b1_sb[:, l, m:m + 1])
                    w2t = w2pool.tile([128, D], BF, tag="w2k")
                    nc.sync.dma_start(
                        w2t[:], w2[l].rearrange("(ko p) d -> p ko d", p=128)[:, m, :])
                    for tt in range(2):
                        nc.tensor.matmul(x2ps[tt][:], gt[:, ds(tt * 128, 128)],
                                         w2t[:], start=(m == 0), stop=(m == MD - 1))
                for tt in range(2):
                    if with_bias:
                        nc.vector.tensor_tensor(x2ps[tt][:], x2ps[tt][:], b2_b[:],
                                                OP.add)
                    nc.vector.tensor_tensor(xt[:, tt, :], xt[:, tt, :], x2ps[tt][:],
                                            OP.add)

              if not with_head:
                  continue
              # ---- final LN + AllGather hidden ----
              xf = layernorm(xt, "h")
              xfT = transpose2(xf, "t4")
              xf_in = dram.tile([XF], BF, tag="xfin")
              nc.sync.dma_start(
                  xf_in[:].rearrange("(p a b) -> p a b", p=128, a=KD), xfT[:])
              xf_out = dram.tile([NC, XF], BF, tag="xfout", addr_space="Shared")
              nc.gpsimd.collective_compute(
                  "AllGather", OP.bypass,
                  replica_groups=[list(range(NC))],
                  ins=[xf_in[:].opt()], outs=[xf_out[:].opt()])
              xfall = []
              for cg in range(2):
                  xa = xftp.tile([128, KD, 4, CH], BF, tag="xft")
                  for c in range(4):
                      nc.sync.dma_start(
                          xa[:, :, c, :],
                          xf_out[cg * 4 + c, :].rearrange("(p a b) -> p a b",
                                                          p=128, a=KD))
                  xfall.append(xa)

              # ---- LM head: vocab-sharded, weight-stationary ----
              # Per 128-vocab block: 4 LDW (one per k) x 4 back-to-back
              # N=512 matmuls into 4 PSUM banks (one per token block);
              # k-accumulation outer so each stationary serves 4 matmuls.
              NWT = VS // 640
              for wt in range(NWT):
                  wl = w4pool.tile([128, KD, 640], BF, tag="w4h")
                  if head_mode != "nowlm" or wt == 0:
                      nc.sync.dma_start(
                          wl[:],
                          wlm.rearrange("(ko p) v -> p ko v", p=128)[:, :, ds(wt * 640, 640)])
                  for j in range(5):
                      vb = wt * 5 + j
                      if vb % 2 == 0:
                          pspec = [(mmp, "mm"), (mmp, "mm"), (avp, "av"),
                                   (avp, "av")]
                      else:
                          pspec = [(spp, "sp"), (spp, "sp"), (mop, "mo"),
                                   (mop, "mo")]
                      banks = [pool_i.tile([128, 512], FP, tag=ptag,
                                           name=f"hb{vb}_{tb}")
                               for tb, (pool_i, ptag) in enumerate(pspec)]
                      for k in range(KD):
                          for tb in range(4):
                              nc.tensor.matmul(
                                  banks[tb][:],
                                  wl[:, k, ds(j * 128, 128)],
                                  xfall[tb // 2][:, k, ds((tb % 2) * 2, 2), :],
                                  start=(k == 0), stop=(k == KD - 1))
                      lgsb = lgp.tile([128, 4, 512], BF, tag="lg",
                                      name=f"lg{vb}")
                      if head_mode != "nocopy":
                          for tb in range(4):
                              if tb % 2 == 0:
                                  nc.scalar.copy(lgsb[:, tb, :], banks[tb][:])
                              else:
                                  nc.vector.tensor_copy(lgsb[:, tb, :],
                                                        banks[tb][:])
                      if head_mode != "nodma":
                          nc.sync.dma_start(
                              logits[ds(vb * 128, 128), :], lgsb[:])
    nc.compile()
    return nc


_CACHE = {}


def _get_program(with_bias=True):
    key = ("nc", with_bias)
    if key not in _CACHE:
        _CACHE[key] = build_program(with_bias=with_bias)
    return _CACHE[key]


def _prep_inputs(inputs):
    f = lambda k: np.asarray(inputs[k], np.float32)
    bf = ml_dtypes.bfloat16
    idx = np.asarray(inputs["idx"]).astype(np.int64)
    tok_emb, pos_emb = f("tok_emb"), f("pos_emb")
    x0 = tok_emb[idx] + pos_emb[None, :T]          # [B, T, D]
    x0 = x0.reshape(NC, CH, D)

    ln1_g, ln1_b = f("ln1_g"), f("ln1_b")
    ln2_g, ln2_b = f("ln2_g"), f("ln2_b")
    Wq, bq = f("Wq"), f("bq")
    Wk, bk = f("Wk"), f("bk")
    Wv, bv = f("Wv"), f("bv")
    Wo, bo = f("Wo"), f("bo")
    W1, b1 = f("W1"), f("b1")
    W2, b2 = f("W2"), f("b2")
    lnf_g, lnf_b = f("lnf_g"), f("lnf_b")
    Wlm, blm = f("Wlm"), f("blm")

    sc = 1.0 / np.sqrt(HS)
    wqe = ln1_g[:, :, None] * Wq * sc
    bqe = (np.einsum("ld,ldm->lm", ln1_b, Wq) + bq) * sc
    wke = ln1_g[:, :, None] * Wk
    bke = np.einsum("ld,ldm->lm", ln1_b, Wk) + bk
    wve = ln1_g[:, :, None] * Wv
    bve = np.einsum("ld,ldm->lm", ln1_b, Wv) + bv
    boe = np.einsum("lm,lmd->ld", bve, Wo) + bo
    w1e = ln2_g[:, :, None] * W1
    b1e = np.einsum("ld,ldf->lf", ln2_b, W1) + b1
    wlme = lnf_g[:, None] * Wlm
    blme = lnf_b @ Wlm + blm

    bqk = np.stack([bqe, bke], axis=1)             # [L, 2, D]
    bqk = bqk.reshape(L, 2, KD, 128).transpose(3, 0, 1, 2).copy()
    b1t = b1e.reshape(L, MD, 128).transpose(2, 0, 1).copy()
    bo2 = np.stack([boe, b2], axis=1)              # [L, 2, D]

    wlmp = np.zeros((D, NC * VS), np.float32)
    wlmp[:, :V] = wlme

    # causal 0/1 masks per core (multiplied in after exp): [p, kc, kt, q]
    masks = []
    for core in range(NC):
        cc = core % 4
        qpos = cc * CH + np.arange(CH)
        m = np.empty((128, 4, 2, CH), np.float32)
        for kc in range(4):
            for kt in range(2):
                kpos = kc * CH + kt * 128 + np.arange(128)
                m[:, kc, kt, :] = (kpos[:, None] <= qpos[None, :]).astype(np.float32)
        masks.append(m.astype(bf))

    shared = dict(wq=np.ascontiguousarray(wqe.astype(bf)),
                  wk=np.ascontiguousarray(wke.astype(bf)),
                  wv=np.ascontiguousarray(wve.astype(bf)),
                  wo=np.ascontiguousarray(Wo.astype(bf)),
                  w1=np.ascontiguousarray(w1e.astype(bf)),
                  w2=np.ascontiguousarray(W2.astype(bf)),
                  bqk=bqk, b1t=b1t, bo2=np.ascontiguousarray(bo2))
    in_maps = []
    for core in range(NC):
        m = dict(shared)
        m["x0"] = np.ascontiguousarray(x0[core])
        m["msk"] = masks[core]
        m["wlm"] = np.ascontiguousarray(
            wlmp[:, core * VS:(core + 1) * VS].astype(bf))
        in_maps.append(m)
    return in_maps, blme


def _run(inputs, trace=False):
    in_maps, blme = _prep_inputs(inputs)
    with_bias = bool(np.any(in_maps[0]["bo2"]))
    nc = _get_program(with_bias=with_bias)
    res = bass_utils.run_bass_kernel_spmd(nc, in_maps, core_ids=list(range(NC)),
                                          trace=trace)
    lg = np.concatenate([np.asarray(res.results[c]["logits"], np.float32)
                         for c in range(NC)], axis=0)   # [NC*VS, B*T]
    out = lg[:V, :].T
    if np.any(blme):
        out = out + blme[None, :]
    return out.reshape(B, T, V).astype(np.float32), res


def kernel(**inputs) -> np.ndarray:
    out, _ = _run(inputs, trace=False)
    return out


# revision 13
# speedup vs baseline: 2.5713x; 1.0412x over previous
"""Trainium2 Bass kernel for a 6-layer GPT (D=512, H=8, T=1024, B=2, V=50257).

Strategy (8 NeuronCores):
- Token-shard the transformer body: core c owns 256 tokens (cores 0-3 =
  batch 0 chunks 0-3, cores 4-7 = batch 1 chunks 0-3).
- All matmuls in bf16 (weights pre-cast host-side, activations cast on
  write); PSUM accumulation stays fp32; residual stream + LN stats fp32.
- Per layer: LN1 -> QKV (q/k in transposed [dim, tok] layout, v natural)
  -> AllGather K,V (bf16, Shared output) across the 4-core batch group
  -> causal attention in transposed layout; AV accumulates directly in
  PSUM across key chunks (2 heads in flight), softmax denominator fused
  as a ones-row of V; approx-reciprocal normalize -> Wo + residual ->
  LN2 -> MLP -> residual.
- Final LN -> AllGather hidden (bf16) across all 8 cores -> vocab-
  sharded LM head in bf16; logits stored bf16 and upcast host-side.
"""

import numpy as np
import ml_dtypes

import concourse.bass as bass
import concourse.tile as tile
from concourse import bacc, mybir
from concourse import bass_utils
from concourse.bass import ds, ts
from concourse.masks import make_identity

FP = mybir.dt.float32
BF = mybir.dt.bfloat16
AF = mybir.ActivationFunctionType
OP = mybir.AluOpType

V, D, T, L, H, HS, B = 50257, 512, 1024, 6, 8, 64, 2
FF = 4 * D
EPS = 1e-5
NC = 8          # cores
CH = 256        # tokens per core
VS = 6400       # padded vocab shard per core; 8*VS = 51200 >= V
KD = D // 128   # 4 k-tiles over D
MD = FF // 128  # 16 m-tiles over FF


def build_program(reps=1, with_bias=True, layers=L, with_head=True, with_attn=True, head_mode='full'):
    nc = bacc.Bacc("TRN2", target_bir_lowering=False, debug=False, num_devices=NC)

    # ---- I/O ----
    x0 = nc.dram_tensor("x0", [CH, D], FP, kind="ExternalInput").ap()
    wq = nc.dram_tensor("wq", [L, D, D], BF, kind="ExternalInput").ap()
    wk = nc.dram_tensor("wk", [L, D, D], BF, kind="ExternalInput").ap()
    wv = nc.dram_tensor("wv", [L, D, D], BF, kind="ExternalInput").ap()
    wo = nc.dram_tensor("wo", [L, D, D], BF, kind="ExternalInput").ap()
    w1 = nc.dram_tensor("w1", [L, D, FF], BF, kind="ExternalInput").ap()
    w2 = nc.dram_tensor("w2", [L, FF, D], BF, kind="ExternalInput").ap()
    wlm = nc.dram_tensor("wlm", [D, VS], BF, kind="ExternalInput").ap()
    # bqk[p, l, 0/1, kd]: per-partition bias for qT/kT ([D] rearranged)
    bqk = nc.dram_tensor("bqk", [128, L, 2, KD], FP, kind="ExternalInput").ap()
    b1t = nc.dram_tensor("b1t", [128, L, MD], FP, kind="ExternalInput").ap()
    # bo2[l, 0]=bo_eff, [l, 1]=b2 (free-dim biases, broadcast via DMA)
    bo2 = nc.dram_tensor("bo2", [L, 2, D], FP, kind="ExternalInput").ap()
    # causal mask per core: [p, kchunk, ktile, 256 queries]
    msk = nc.dram_tensor("msk", [128, 4, 2, CH], BF, kind="ExternalInput").ap()
    # transposed logits: rows = vocab shard, cols = tokens (contiguous DMA)
    logits = nc.dram_tensor("logits", [VS, B * T], BF, kind="ExternalOutput").ap()

    KVH_K = 128 * KD * 128          # kT half-block flat size per core
    KVH_V = 128 * H * 65            # v_aug half-block flat size per core
    KVH = KVH_K + KVH_V
    XF = 128 * KD * CH              # xfT flat size

    from contextlib import ExitStack
    with ExitStack() as stk:
        tc = stk.enter_context(tile.TileContext(nc))
        ec = stk.enter_context
        consts = ec(tc.tile_pool(name="consts", bufs=1))
        xpool = ec(tc.tile_pool(name="xpool", bufs=1))
        hpool = ec(tc.tile_pool(name="hpool", bufs=2))
        t4 = ec(tc.tile_pool(name="t4", bufs=4))          # [128,KD,CH] transposed acts
        w4pool = ec(tc.tile_pool(name="w4", bufs=4))      # [128,KD,512] weights
        w1pool = ec(tc.tile_pool(name="w1k", bufs=4))     # [128,FF] W1 k-tiles
        w2pool = ec(tc.tile_pool(name="w2k", bufs=4))     # [128,512] W2 k-tiles
        kvall = ec(tc.tile_pool(name="kvall", bufs=1))
        vaugp = ec(tc.tile_pool(name="vaug", bufs=1))
        small = ec(tc.tile_pool(name="small", bufs=2))
        expp = ec(tc.tile_pool(name="exp", bufs=3))
        gtp = ec(tc.tile_pool(name="gt", bufs=2))
        lgp = ec(tc.tile_pool(name="lg", bufs=2))
        bcp = ec(tc.tile_pool(name="bcast", bufs=2))
        xftp = ec(tc.tile_pool(name="xft", bufs=2))
        mmp = ec(tc.tile_pool(name="mm", bufs=2, space="PSUM"))
        avp = ec(tc.tile_pool(name="avp", bufs=4, space="PSUM"))
        spp = ec(tc.tile_pool(name="sp", bufs=2, space="PSUM"))
        avacc = ec(tc.tile_pool(name="avacc", bufs=1))
        dram = ec(tc.tile_pool(name="dram", bufs=2, space="DRAM"))
        if True:
            ident = consts.tile([128, 128], BF)
            make_identity(nc, ident)
            epst = consts.tile([128, 1], FP)
            nc.vector.memset(epst, EPS)
            bqk_sb = consts.tile([128, L, 2, KD], FP)
            nc.sync.dma_start(bqk_sb[:], bqk)
            b1_sb = consts.tile([128, L, MD], FP)
            nc.sync.dma_start(b1_sb[:], b1t)
            msk_sb = consts.tile([128, 4, 2, CH], BF)
            nc.sync.dma_start(msk_sb[:], msk)

            def layernorm(src, tag):
                """src [128,2,D] fp32 -> fresh normalized bf16 tile (no affine)."""
                out = hpool.tile([128, 2, D], BF, tag="h")
                for tt in range(2):
                    st = small.tile([128, 6], FP, tag="bnst")
                    nc.vector.bn_stats(st[:], src[:, tt, :])
                    mv = small.tile([128, 2], FP, tag="bnmv")
                    nc.vector.bn_aggr(mv[:], st[:])
                    nc.scalar.activation(mv[:, 1:2], mv[:, 1:2], AF.Sqrt,
                                         bias=epst[:, 0:1])
                    nc.vector.reciprocal(mv[:, 1:2], mv[:, 1:2])
                    nc.vector.tensor_scalar(
                        out=out[:, tt, :], in0=src[:, tt, :],
                        scalar1=mv[:, 0:1], scalar2=mv[:, 1:2],
                        op0=OP.subtract, op1=OP.mult)
                return out

            def transpose2(src, tag):
                """src [128,2,D] bf16 (tokens, dims) -> [128,KD,CH] (dims, tokens)."""
                out = t4.tile([128, KD, CH], BF, tag="t4")
                for d in range(KD):
                    for tt in range(2):
                        ps = mmp.tile([128, 128], BF, tag="mm")
                        nc.tensor.transpose(ps[:], src[:, tt, ds(d * 128, 128)],
                                            ident[:])
                        nc.vector.tensor_copy(out[:, d, ds(tt * 128, 128)],
                                              ps[:])
                return out

            def load_w4(src_ap, tag="w4"):
                w = w4pool.tile([128, KD, 512], BF, tag=tag)
                nc.sync.dma_start(w[:], src_ap.rearrange("(ko p) m -> p ko m", p=128))
                return w

            for rep in range(reps):
              xt = xpool.tile([128, 2, D], FP, tag="xt", name=f"xt{rep}")
              nc.sync.dma_start(xt[:], x0.rearrange("(tt p) d -> p tt d", p=128))
              for l in range(layers):
                # ---- LN1 + transpose ----
                h = layernorm(xt, "h")
                hT = transpose2(h, "t4")

                # ---- k/v for token block A, AllGather A starts ASAP ----
                wk_sb = load_w4(wk[l])
                wv_sb = load_w4(wv[l])
                kT = t4.tile([128, KD, CH], BF, tag="t4")
                vaug = vaugp.tile([128, 2, H, 65], BF)
                nc.vector.memset(vaug[:, :, :, 64:65], 1.0)
                kvh_in = []
                kvh_out = []
                for tt in range(2):
                    for d in range(KD):
                        ps = mmp.tile([128, 512], FP, tag="mm")
                        for k in range(KD):
                            nc.tensor.matmul(
                                ps[:, :128],
                                wk_sb[:, k, ds(d * 128, 128)],
                                hT[:, k, ds(tt * 128, 128)], start=(k == 0),
                                stop=(k == KD - 1))
                        nc.vector.tensor_scalar_add(kT[:, d, ds(tt * 128, 128)],
                                                    ps[:, :128],
                                                    bqk_sb[:, l, 1, d:d + 1])
                    ps = mmp.tile([128, 512], FP, tag="mm")
                    for k in range(KD):
                        nc.tensor.matmul(ps[:, :D], hT[:, k, ds(tt * 128, 128)],
                                         wv_sb[:, k, :], start=(k == 0),
                                         stop=(k == KD - 1))
                    for hh in range(H):
                        nc.vector.tensor_copy(vaug[:, tt, hh, 0:64],
                                              ps[:, ds(hh * 64, 64)])
                    # half-payload allgather for this token block
                    kin = dram.tile([KVH], BF, tag=f"kvin{tt}")
                    nc.sync.dma_start(
                        kin[0:KVH_K].rearrange("(p a b) -> p a b", p=128, a=KD),
                        kT[:, :, ds(tt * 128, 128)])
                    nc.sync.dma_start(
                        kin[KVH_K:KVH].rearrange("(p h e) -> p h e", p=128, h=H),
                        vaug[:, tt])
                    kout = dram.tile([4, KVH], BF, tag=f"kvout{tt}")
                    nc.gpsimd.collective_compute(
                        "AllGather", OP.bypass,
                        replica_groups=[[0, 1, 2, 3], [4, 5, 6, 7]],
                        ins=[kin[:].opt()], outs=[kout[:].opt()])
                    kvh_in.append(kin)
                    kvh_out.append(kout)

                # ---- q projection overlaps the collectives ----
                wq_sb = load_w4(wq[l])
                qT = t4.tile([128, KD, CH], BF, tag="t4")
                for d in range(KD):
                    ps = mmp.tile([128, 512], FP, tag="mm")
                    for k in range(KD):
                        nc.tensor.matmul(ps[:, :CH], wq_sb[:, k, ds(d * 128, 128)],
                                         hT[:, k, :], start=(k == 0),
                                         stop=(k == KD - 1))
                    nc.vector.tensor_scalar_add(qT[:, d, :], ps[:, :CH],
                                                bqk_sb[:, l, 0, d:d + 1])

                if not with_attn:
                    continue
                kTall = kvall.tile([128, KD, 4, CH], BF, tag="ktall")
                vall = kvall.tile([128, 4, 2, H, 65], BF, tag="vall")
                for kt in range(2):
                    for c in range(4):
                        nc.sync.dma_start(
                            kTall[:, :, c, ds(kt * 128, 128)],
                            kvh_out[kt][c, 0:KVH_K].rearrange(
                                "(p a b) -> p a b", p=128, a=KD))
                        nc.sync.dma_start(
                            vall[:, c, kt, :, :],
                            kvh_out[kt][c, KVH_K:KVH].rearrange(
                                "(p h e) -> p h e", p=128, h=H))

                # ---- attention (transposed layout) ----
                # AV accumulates in PSUM; 4 heads in flight per pass
                # (4 PSUM banks), 2 passes.  kt (key token block) is the
                # outer loop so kt=0 work overlaps the block-B AllGather.
                denrow = avacc.tile([65, H, CH], FP, tag="denrow")
                attn_nb = avacc.tile([64, H, CH], BF, tag="attnb")
                for pas in range(2):
                    hds = tuple(range(4 * pas, 4 * pas + 4))
                    av_ps = {}
                    for hh in hds:
                        av_ps[hh] = avp.tile([65, CH], FP, tag="av",
                                             name=f"av{l}_{hh}")
                    for kt in range(2):
                        for c in range(4):
                            for hh in hds:
                                pb = (hh % 2) * 64
                                dt_ = hh // 2
                                sps = spp.tile([128, CH], FP, tag="sp")
                                nc.tensor.matmul(
                                    sps[:, :CH],
                                    kTall[pb:pb + 64, dt_, c, ds(kt * 128, 128)],
                                    qT[pb:pb + 64, dt_, :],
                                    start=True, stop=True)
                                ex = expp.tile([128, CH], BF, tag="exp")
                                nc.scalar.activation(ex[:], sps[:, :CH], AF.Exp)
                                nc.gpsimd.tensor_tensor(ex[:], ex[:],
                                                        msk_sb[:, c, kt, :],
                                                        OP.mult)
                                nc.tensor.matmul(
                                    av_ps[hh][:], vall[:, c, kt, hh, :], ex[:],
                                    start=(c == 0 and kt == 0),
                                    stop=(c == 3 and kt == 1))
                    # ---- normalize these 4 heads (overlaps next pass) ----
                    for hh in hds:
                        nc.vector.tensor_copy(denrow[64:65, hh, :],
                                              av_ps[hh][64:65, :])
                    rdram = dram.tile([4, CH], FP, tag="rdram",
                                      name=f"rd{l}_{pas}")
                    nc.sync.dma_start(rdram[:], denrow[64:65, ds(4 * pas, 4), :])
                    rbc = avacc.tile([64, 4, CH], FP, tag="rbc")
                    for i in range(4):
                        nc.sync.dma_start(
                            rbc[:, i, :],
                            bass.AP(tensor=rdram.tensor,
                                    offset=rdram.offset + i * CH,
                                    ap=[[0, 64], [1, CH]]))
                    nc.vector.reciprocal_approx_fast(rbc[:], rbc[:])
                    for i, hh in enumerate(hds):
                        nc.vector.tensor_tensor(attn_nb[:, hh, :],
                                                av_ps[hh][0:64, :], rbc[:, i, :],
                                                OP.mult)
                attT = t4.tile([128, KD, CH], BF, tag="t4")
                for dt_ in range(KD):
                    for a in range(2):
                        nc.sync.dma_start(attT[a * 64:(a + 1) * 64, dt_, :],
                                          attn_nb[:, 2 * dt_ + a, :])

                # ---- Wo + bias + residual ----
                wo_sb = load_w4(wo[l])
                if with_bias:
                    bo_b = bcp.tile([128, D], FP, tag="bc")
                    bo_src = bo2[l, 0]
                    nc.sync.dma_start(bo_b[:], bass.AP(
                        tensor=bo_src.tensor, offset=bo_src.offset,
                        ap=[[0, 128]] + list(bo_src.ap)))
                for tt in range(2):
                    ps = mmp.tile([128, 512], FP, tag="mm")
                    for k in range(KD):
                        nc.tensor.matmul(ps[:, :D], attT[:, k, ds(tt * 128, 128)],
                                         wo_sb[:, k, :], start=(k == 0),
                                         stop=(k == KD - 1))
                    if with_bias:
                        nc.vector.tensor_tensor(ps[:, :D], ps[:, :D], bo_b[:],
                                                OP.add)
                    nc.vector.tensor_tensor(xt[:, tt, :], xt[:, tt, :], ps[:, :D],
                                            OP.add)

                # ---- LN2 + transpose ----
                h2 = layernorm(xt, "h")
                h2T = transpose2(h2, "t4")

                # ---- MLP ----
                w1_sb = [w1pool.tile([128, FF], BF, tag="w1k", name=f"w1k{l}_{kk}") for kk in range(KD)]
                for k in range(KD):
                    nc.sync.dma_start(
                        w1_sb[k][:],
                        w1[l].rearrange("(ko p) f -> p ko f", p=128)[:, k, :])
                if with_bias:
                    b2_b = bcp.tile([128, D], FP, tag="bc")
                    b2_src = bo2[l, 1]
                    nc.sync.dma_start(b2_b[:], bass.AP(
                        tensor=b2_src.tensor, offset=b2_src.offset,
                        ap=[[0, 128]] + list(b2_src.ap)))
                x2ps = [avp.tile([128, D], FP, tag="av", name=f"mo{l}_{kk}") for kk in range(2)]
                for m in range(MD):
                    gps = mmp.tile([128, 512], FP, tag="mm")
                    for k in range(KD):
                        nc.tensor.matmul(gps[:, :CH], w1_sb[k][:, ds(m * 128, 128)],
                                         h2T[:, k, :], start=(k == 0),
                                         stop=(k == KD - 1))
                    gt = gtp.tile([128, CH], BF, tag="gt")
                    nc.scalar.activation(gt[:], gps[:, :CH], AF.Gelu,
                                         bias=b1_sb[:, l, m:m + 1])
                    w2t = w2pool.tile([128, D], BF, tag="w2k")
                    nc.sync.dma_start(
                        w2t[:], w2[l].rearrange("(ko p) d -> p ko d", p=128)[:, m, :])
                    for tt in range(2):
                        nc.tensor.matmul(x2ps[tt][:], gt[:, ds(tt * 128, 128)],
                                         w2t[:], start=(m == 0), stop=(m == MD - 1))
                for tt in range(2):
                    if with_bias:
                        nc.vector.tensor_tensor(x2ps[tt][:], x2ps[tt][:], b2_b[:],
                                                OP.add)
                    nc.vector.tensor_tensor(xt[:, tt, :], xt[:, tt, :], x2ps[tt][:],
                                            OP.add)

              if not with_head:
                  continue
              # ---- final LN + AllGather hidden ----
              xf = layernorm(xt, "h")
              xfT = transpose2(xf, "t4")
              xf_in = dram.tile([XF], BF, tag="xfin")
              nc.sync.dma_start(
                  xf_in[:].rearrange("(p a b) -> p a b", p=128, a=KD), xfT[:])
              xf_out = dram.tile([NC, XF], BF, tag="xfout", addr_space="Shared")
              nc.gpsimd.collective_compute(
                  "AllGather", OP.bypass,
                  replica_groups=[list(range(NC))],
                  ins=[xf_in[:].opt()], outs=[xf_out[:].opt()])
              xfall = []
              for cg in range(2):
                  xa = xftp.tile([128, KD, 4, CH], BF, tag="xft")
                  for c in range(4):
                      nc.sync.dma_start(
                          xa[:, :, c, :],
                          xf_out[cg * 4 + c, :].rearrange("(p a b) -> p a b",
                                                          p=128, a=KD))
                  xfall.append(xa)

              # ---- LM head: vocab-sharded, weight-stationary ----
              # Per 128-vocab block: 4 LDW (one per k) x 4 back-to-back
              # N=512 matmuls into 4 PSUM banks (one per token block);
              # k-accumulation outer so each stationary serves 4 matmuls.
              NWT = VS // 640
              for wt in range(NWT):
                  wl = w4pool.tile([128, KD, 640], BF, tag="w4h")
                  if head_mode != "nowlm" or wt == 0:
                      nc.sync.dma_start(
                          wl[:],
                          wlm.rearrange("(ko p) v -> p ko v", p=128)[:, :, ds(wt * 640, 640)])
                  for j in range(5):
                      vb = wt * 5 + j
                      if vb % 2 == 0:
                          pspec = [(mmp, "mm"), (mmp, "mm"), (spp, "sp"),
                                   (spp, "sp")]
                      else:
                          pspec = [(avp, "av"), (avp, "av"), (avp, "av"),
                                   (avp, "av")]
                      banks = [pool_i.tile([128, 512], FP, tag=ptag,
                                           name=f"hb{vb}_{tb}")
                               for tb, (pool_i, ptag) in enumerate(pspec)]
                      for k in range(KD):
                          for tb in range(4):
                              nc.tensor.matmul(
                                  banks[tb][:],
                                  wl[:, k, ds(j * 128, 128)],
                                  xfall[tb // 2][:, k, ds((tb % 2) * 2, 2), :],
                                  start=(k == 0), stop=(k == KD - 1))
                      lgsb = lgp.tile([128, 4, 512], BF, tag="lg",
                                      name=f"lg{vb}")
                      if head_mode != "nocopy":
                          for tb in range(4):
                              if tb % 2 == 0:
                                  nc.scalar.copy(lgsb[:, tb, :], banks[tb][:])
                              else:
                                  nc.vector.tensor_copy(lgsb[:, tb, :],
                                                        banks[tb][:])
                      if head_mode != "nodma":
                          nc.sync.dma_start(
                              logits[ds(vb * 128, 128), :], lgsb[:])
    nc.compile()
    return nc


_CACHE = {}


def _get_program(with_bias=True):
    key = ("nc", with_bias)
    if key not in _CACHE:
        _CACHE[key] = build_program(with_bias=with_bias)
    return _CACHE[key]


def _prep_inputs(inputs):
    f = lambda k: np.asarray(inputs[k], np.float32)
    bf = ml_dtypes.bfloat16
    idx = np.asarray(inputs["idx"]).astype(np.int64)
    tok_emb, pos_emb = f("tok_emb"), f("pos_emb")
    x0 = tok_emb[idx] + pos_emb[None, :T]          # [B, T, D]
    x0 = x0.reshape(NC, CH, D)

    ln1_g, ln1_b = f("ln1_g"), f("ln1_b")
    ln2_g, ln2_b = f("ln2_g"), f("ln2_b")
    Wq, bq = f("Wq"), f("bq")
    Wk, bk = f("Wk"), f("bk")
    Wv, bv = f("Wv"), f("bv")
    Wo, bo = f("Wo"), f("bo")
    W1, b1 = f("W1"), f("b1")
    W2, b2 = f("W2"), f("b2")
    lnf_g, lnf_b = f("lnf_g"), f("lnf_b")
    Wlm, blm = f("Wlm"), f("blm")

    sc = 1.0 / np.sqrt(HS)
    wqe = ln1_g[:, :, None] * Wq * sc
    bqe = (np.einsum("ld,ldm->lm", ln1_b, Wq) + bq) * sc
    wke = ln1_g[:, :, None] * Wk
    bke = np.einsum("ld,ldm->lm", ln1_b, Wk) + bk
    wve = ln1_g[:, :, None] * Wv
    bve = np.einsum("ld,ldm->lm", ln1_b, Wv) + bv
    boe = np.einsum("lm,lmd->ld", bve, Wo) + bo
    w1e = ln2_g[:, :, None] * W1
    b1e = np.einsum("ld,ldf->lf", ln2_b, W1) + b1
    wlme = lnf_g[:, None] * Wlm
    blme = lnf_b @ Wlm + blm

    bqk = np.stack([bqe, bke], axis=1)             # [L, 2, D]
    bqk = bqk.reshape(L, 2, KD, 128).transpose(3, 0, 1, 2).copy()
    b1t = b1e.reshape(L, MD, 128).transpose(2, 0, 1).copy()
    bo2 = np.stack([boe, b2], axis=1)              # [L, 2, D]

    wlmp = np.zeros((D, NC * VS), np.float32)
    wlmp[:, :V] = wlme

    # causal 0/1 masks per core (multiplied in after exp): [p, kc, kt, q]
    masks = []
    for core in range(NC):
        cc = core % 4
        qpos = cc * CH + np.arange(CH)
        m = np.empty((128, 4, 2, CH), np.float32)
        for kc in range(4):
            for kt in range(2):
                kpos = kc * CH + kt * 128 + np.arange(128)
                m[:, kc, kt, :] = (kpos[:, None] <= qpos[None, :]).astype(np.float32)
        masks.append(m.astype(bf))

    shared = dict(wq=np.ascontiguousarray(wqe.astype(bf)),
                  wk=np.ascontiguousarray(wke.astype(bf)),
                  wv=np.ascontiguousarray(wve.astype(bf)),
                  wo=np.ascontiguousarray(Wo.astype(bf)),
                  w1=np.ascontiguousarray(w1e.astype(bf)),
                  w2=np.ascontiguousarray(W2.astype(bf)),
                  bqk=bqk, b1t=b1t, bo2=np.ascontiguousarray(bo2))
    in_maps = []
    for core in range(NC):
        m = dict(shared)
        m["x0"] = np.ascontiguousarray(x0[core])
        m["msk"] = masks[core]
        m["wlm"] = np.ascontiguousarray(
            wlmp[:, core * VS:(core + 1) * VS].astype(bf))
        in_maps.append(m)
    return in_maps, blme


def _run(inputs, trace=False):
    in_maps, blme = _prep_inputs(inputs)
    with_bias = bool(np.any(in_maps[0]["bo2"]))
    nc = _get_program(with_bias=with_bias)
    res = bass_utils.run_bass_kernel_spmd(nc, in_maps, core_ids=list(range(NC)),
                                          trace=trace)
    lg = np.concatenate([np.asarray(res.results[c]["logits"], np.float32)
                         for c in range(NC)], axis=0)   # [NC*VS, B*T]
    out = lg[:V, :].T
    if np.any(blme):
        out = out + blme[None, :]
    return out.reshape(B, T, V).astype(np.float32), res


def kernel(**inputs) -> np.ndarray:
    out, _ = _run(inputs, trace=False)
    return out


# revision 17
# speedup vs baseline: 2.6766x; 1.0409x over previous
"""Trainium2 Bass kernel for a 6-layer GPT (D=512, H=8, T=1024, B=2, V=50257).

Strategy (8 NeuronCores):
- Token-shard the body with a causal-balanced layout: within each 4-core
  batch group, core r owns global 128-token half-chunks r ("qA") and 7-r
  ("qB"), so every core computes exactly 12 of 16 causal score blocks:
  qB x A_c (unmasked), qA x A_c (masked), qB x B_c (masked); qA x B_c is
  never visible and is skipped uniformly.
- All matmuls bf16; PSUM fp32; residual + LN stats fp32.
- Per layer: LN1 -> K/V for block A -> AllGather-A -> K/V block B ->
  AllGather-B -> Q -> attention: A-unit blocks (overlapping AllGather-B)
  then B-unit blocks; AV accumulates in PSUM, 4 heads in flight; odd
  heads use a [0|1|v] padded V stationary so their AV lands in
  partitions 64-127 directly (no transpose shuffle); softmax denominator
  fused as a ones column; approx-reciprocal normalize -> Wo + residual
  -> LN2 -> MLP -> residual.
- Final LN -> 8-core AllGather -> vocab-sharded LM head, weight-
  stationary (4 token-block matmuls per LDWEIGHTS, 4 PSUM banks),
  logits written transposed [VS, B*T] in bf16, host unpermutes/upcasts.
"""

import numpy as np
import ml_dtypes

import concourse.bass as bass
import concourse.tile as tile
from concourse import bacc, mybir
from concourse import bass_utils
from concourse.bass import ds, ts
from concourse.masks import make_identity

FP = mybir.dt.float32
BF = mybir.dt.bfloat16
AF = mybir.ActivationFunctionType
OP = mybir.AluOpType

V, D, T, L, H, HS, B = 50257, 512, 1024, 6, 8, 64, 2
FF = 4 * D
EPS = 1e-5
NC = 8          # cores
CH = 256        # tokens per core
VS = 6400       # padded vocab shard per core; 8*VS = 51200 >= V
KD = D // 128   # 4 k-tiles over D
MD = FF // 128  # 16 m-tiles over FF

# v slot order in vall2: even heads first, then odd (parity, dt)
VSLOT = {hh: (hh % 2) * 4 + hh // 2 for hh in range(H)}


def build_program(reps=1, with_bias=True, layers=L, with_head=True, with_attn=True, head_mode='full'):
    nc = bacc.Bacc("TRN2", target_bir_lowering=False, debug=False, num_devices=NC)

    # ---- I/O ----
    x0 = nc.dram_tensor("x0", [CH, D], FP, kind="ExternalInput").ap()
    wq = nc.dram_tensor("wq", [L, D, D], BF, kind="ExternalInput").ap()
    wk = nc.dram_tensor("wk", [L, D, D], BF, kind="ExternalInput").ap()
    wv = nc.dram_tensor("wv", [L, D, D], BF, kind="ExternalInput").ap()
    wo = nc.dram_tensor("wo", [L, D, D], BF, kind="ExternalInput").ap()
    w1 = nc.dram_tensor("w1", [L, D, FF], BF, kind="ExternalInput").ap()
    w2 = nc.dram_tensor("w2", [L, FF, D], BF, kind="ExternalInput").ap()
    wlm = nc.dram_tensor("wlm", [D, VS], BF, kind="ExternalInput").ap()
    # bqk[p, l, 0/1, kd]: per-partition bias for qT/kT ([D] rearranged)
    bqk = nc.dram_tensor("bqk", [128, L, 2, KD], FP, kind="ExternalInput").ap()
    b1t = nc.dram_tensor("b1t", [128, L, MD], FP, kind="ExternalInput").ap()
    # bo2[l, 0]=bo_eff, [l, 1]=b2 (free-dim biases, broadcast via DMA)
    bo2 = nc.dram_tensor("bo2", [L, 2, D], FP, kind="ExternalInput").ap()
    # causal masks: [p, 0, c, q]=qA vs A_c ; [p, 1, c, q]=qB vs B_c
    msk = nc.dram_tensor("msk", [128, 2, 4, 128], BF, kind="ExternalInput").ap()
    # transposed logits: rows = vocab shard, cols = tokens (contiguous DMA)
    logits = nc.dram_tensor("logits", [VS, B * T], BF, kind="ExternalOutput").ap()

    KVH_K = 128 * KD * 128          # kT half-block flat size per core
    KVH_V = 128 * H * 65            # v_aug half-block flat size per core
    KVH = KVH_K + KVH_V
    XF = 128 * KD * CH              # xfT flat size

    from contextlib import ExitStack
    with ExitStack() as stk:
        tc = stk.enter_context(tile.TileContext(nc))
        ec = stk.enter_context
        consts = ec(tc.tile_pool(name="consts", bufs=1))
        persist = ec(tc.tile_pool(name="persist", bufs=1))
        xpool = ec(tc.tile_pool(name="xpool", bufs=1))
        hpool = ec(tc.tile_pool(name="hpool", bufs=2))
        t4 = ec(tc.tile_pool(name="t4", bufs=4))          # [128,KD,CH] transposed acts
        w4pool = ec(tc.tile_pool(name="w4", bufs=4))      # [128,KD,512] weights
        w1pool = ec(tc.tile_pool(name="w1k", bufs=1))     # [128,KD,FF] W1
        w2pool = ec(tc.tile_pool(name="w2k", bufs=1))     # [128,MD,512] W2
        vaugp = ec(tc.tile_pool(name="vaug", bufs=1))
        small = ec(tc.tile_pool(name="small", bufs=2))
        expp = ec(tc.tile_pool(name="exp", bufs=4))
        gtp = ec(tc.tile_pool(name="gt", bufs=2))
        lgp = ec(tc.tile_pool(name="lg", bufs=2))
        bcp = ec(tc.tile_pool(name="bcast", bufs=2))
        xftp = ec(tc.tile_pool(name="xft", bufs=2))
        mmp = ec(tc.tile_pool(name="mm", bufs=2, space="PSUM"))
        avp = ec(tc.tile_pool(name="avp", bufs=4, space="PSUM"))
        spp = ec(tc.tile_pool(name="sp", bufs=2, space="PSUM"))
        avacc = ec(tc.tile_pool(name="avacc", bufs=2))
        dram = ec(tc.tile_pool(name="dram", bufs=2, space="DRAM"))
        if True:
            ident = consts.tile([128, 128], BF)
            make_identity(nc, ident)
            epst = consts.tile([128, 1], FP)
            nc.vector.memset(epst, EPS)
            bqk_sb = consts.tile([128, L, 2, KD], FP)
            nc.sync.dma_start(bqk_sb[:], bqk)
            b1_sb = consts.tile([128, L, MD], FP)
            nc.sync.dma_start(b1_sb[:], b1t)
            msk_sb = consts.tile([128, 2, 4, 128], BF)
            nc.sync.dma_start(msk_sb[:], msk)

            # persistent attention buffers (written fresh each layer)
            kTall = persist.tile([128, KD, 4, CH], BF, tag="ktall")
            # vall2: slots 0:4 = even heads (cols 0:65 = v|1), slots 4:8 =
            # odd heads (col 0 = ones -> den row 0, cols 64:128 = v,
            # cols 1:64 zeroed once here)
            vall2 = persist.tile([128, 4, 2, 8, 128], BF, tag="vall")
            nc.vector.memset(vall2[:, :, :, 4:8, 1:64], 0.0)
            attn_nb2 = persist.tile([128, KD, CH], BF, tag="attnb")
            denrow = persist.tile([65, H, CH], FP, tag="denrow")

            def layernorm(src, tag):
                """src [128,2,D] fp32 -> fresh normalized bf16 tile (no affine)."""
                out = hpool.tile([128, 2, D], BF, tag="h")
                for tt in range(2):
                    st = small.tile([128, 6], FP, tag="bnst")
                    nc.vector.bn_stats(st[:], src[:, tt, :])
                    mv = small.tile([128, 2], FP, tag="bnmv")
                    nc.vector.bn_aggr(mv[:], st[:])
                    nc.scalar.activation(mv[:, 1:2], mv[:, 1:2], AF.Sqrt,
                                         bias=epst[:, 0:1])
                    nc.vector.reciprocal(mv[:, 1:2], mv[:, 1:2])
                    nc.vector.tensor_scalar(
                        out=out[:, tt, :], in0=src[:, tt, :],
                        scalar1=mv[:, 0:1], scalar2=mv[:, 1:2],
                        op0=OP.subtract, op1=OP.mult)
                return out

            def transpose2(src, tag):
                """src [128,2,D] bf16 (tokens, dims) -> [128,KD,CH] (dims, tokens)."""
                out = t4.tile([128, KD, CH], BF, tag="t4")
                for d in range(KD):
                    for tt in range(2):
                        ps = mmp.tile([128, 128], BF, tag="mm")
                        nc.tensor.transpose(ps[:], src[:, tt, ds(d * 128, 128)],
                                            ident[:])
                        nc.vector.tensor_copy(out[:, d, ds(tt * 128, 128)],
                                              ps[:])
                return out

            def load_w4(src_ap, tag="w4"):
                w = w4pool.tile([128, KD, 512], BF, tag=tag)
                nc.sync.dma_start(w[:], src_ap.rearrange("(ko p) m -> p ko m", p=128))
                return w

            for rep in range(reps):
              xt = xpool.tile([128, 2, D], FP, tag="xt", name=f"xt{rep}")
              nc.sync.dma_start(xt[:], x0.rearrange("(tt p) d -> p tt d", p=128))
              for l in range(layers):
                # ---- LN1 + transpose ----
                h = layernorm(xt, "h")
                hT = transpose2(h, "t4")

                # ---- k/v per token block; AllGather each block ASAP ----
                wk_sb = load_w4(wk[l])
                wv_sb = load_w4(wv[l])
                kT = t4.tile([128, KD, CH], BF, tag="t4")
                vaug = vaugp.tile([128, 2, H, 65], BF)
                nc.vector.memset(vaug[:, :, :, 64:65], 1.0)
                kvh_out = []
                for tt in range(2):
                    for d in range(KD):
                        ps = mmp.tile([128, 512], FP, tag="mm")
                        for k in range(KD):
                            nc.tensor.matmul(
                                ps[:, :128],
                                wk_sb[:, k, ds(d * 128, 128)],
                                hT[:, k, ds(tt * 128, 128)], start=(k == 0),
                                stop=(k == KD - 1))
                        nc.vector.tensor_scalar_add(kT[:, d, ds(tt * 128, 128)],
                                                    ps[:, :128],
                                                    bqk_sb[:, l, 1, d:d + 1])
                    ps = mmp.tile([128, 512], FP, tag="mm")
                    for k in range(KD):
                        nc.tensor.matmul(ps[:, :D], hT[:, k, ds(tt * 128, 128)],
                                         wv_sb[:, k, :], start=(k == 0),
                                         stop=(k == KD - 1))
                    for hh in range(H):
                        # store with even/odd slot interleave
                        nc.vector.tensor_copy(vaug[:, tt, VSLOT[hh], 0:64],
                                              ps[:, ds(hh * 64, 64)])
                    kin = dram.tile([KVH], BF, tag=f"kvin{tt}")
                    nc.sync.dma_start(
                        kin[0:KVH_K].rearrange("(p a b) -> p a b", p=128, a=KD),
                        kT[:, :, ds(tt * 128, 128)])
                    nc.sync.dma_start(
                        kin[KVH_K:KVH].rearrange("(p h e) -> p h e", p=128, h=H),
                        vaug[:, tt])
                    kout = dram.tile([4, KVH], BF, tag=f"kvout{tt}")
                    nc.gpsimd.collective_compute(
                        "AllGather", OP.bypass,
                        replica_groups=[[0, 1, 2, 3], [4, 5, 6, 7]],
                        ins=[kin[:].opt()], outs=[kout[:].opt()])
                    kvh_out.append(kout)

                # ---- q projection + weight prefetch overlap collectives ----
                wq_sb = load_w4(wq[l])
                qT = t4.tile([128, KD, CH], BF, tag="t4")
                for d in range(KD):
                    ps = mmp.tile([128, 512], FP, tag="mm")
                    for k in range(KD):
                        nc.tensor.matmul(ps[:, :CH], wq_sb[:, k, ds(d * 128, 128)],
                                         hT[:, k, :], start=(k == 0),
                                         stop=(k == KD - 1))
                    nc.vector.tensor_scalar_add(qT[:, d, :], ps[:, :CH],
                                                bqk_sb[:, l, 0, d:d + 1])
                wo_sb = load_w4(wo[l])
                w1_sb = w1pool.tile([128, KD, FF], BF, tag="w1k")
                nc.sync.dma_start(
                    w1_sb[:], w1[l].rearrange("(ko p) f -> p ko f", p=128))
                w2_sb = w2pool.tile([128, MD, 512], BF, tag="w2k")
                nc.sync.dma_start(
                    w2_sb[:], w2[l].rearrange("(ko p) d -> p ko d", p=128))

                if not with_attn:
                    continue
                # ---- unpack gathered K/V halves ----
                for kt in range(2):
                    ko = kvh_out[kt]
                    for c in range(4):
                        nc.sync.dma_start(
                            kTall[:, :, c, ds(kt * 128, 128)],
                            ko[c, 0:KVH_K].rearrange("(p a b) -> p a b",
                                                     p=128, a=KD))
                        src = ko[c, KVH_K:KVH].rearrange("(p h e) -> p h e",
                                                         p=128, h=H)
                        nc.sync.dma_start(vall2[:, c, kt, 0:4, 0:65],
                                          src[:, 0:4, :])
                        nc.sync.dma_start(vall2[:, c, kt, 4:8, 64:128],
                                          src[:, 4:8, 0:64])
                        nc.sync.dma_start(vall2[:, c, kt, 4:8, 0:1],
                                          src[:, 4:8, 64:65])

                # ---- attention: A-unit blocks (both q halves, masked on
                # qA), then B-unit blocks (qB half only). 4 heads per pass
                # accumulate AV in 4 PSUM banks across all 8 blocks. ----
                for pas in range(2):
                    hds = tuple(range(4 * pas, 4 * pas + 4))
                    av_ps = {}
                    for hh in hds:
                        av_ps[hh] = avp.tile([128, CH], FP, tag="av",
                                             name=f"av{l}_{hh}")
                    for kt in range(2):
                        qsl = ds(0, CH) if kt == 0 else ds(128, 128)
                        nq = CH if kt == 0 else 128
                        for c in range(4):
                            for hh in hds:
                                pb = (hh % 2) * 64
                                dt_ = hh // 2
                                sl = VSLOT[hh]
                                sps = spp.tile([128, CH], FP, tag="sp")
                                nc.tensor.matmul(
                                    sps[:, :nq],
                                    kTall[pb:pb + 64, dt_, c, ds(kt * 128, 128)],
                                    qT[pb:pb + 64, dt_, qsl],
                                    start=True, stop=True)
                                ex = expp.tile([128, CH], BF, tag="exp")
                                nc.scalar.activation(ex[:, :nq], sps[:, :nq],
                                                     AF.Exp)
                                # mask: kt=0 -> qA cols only; kt=1 -> all
                                meng = nc.gpsimd if hh % 2 == 0 else nc.vector
                                meng.tensor_tensor(ex[:, 0:128], ex[:, 0:128],
                                                   msk_sb[:, kt, c, :], OP.mult)
                                if hh % 2 == 0:
                                    nc.tensor.matmul(
                                        av_ps[hh][0:65, ds(kt * 128, nq)],
                                        vall2[:, c, kt, sl, 0:65],
                                        ex[:, :nq],
                                        start=(kt == 0 and c == 0),
                                        stop=(kt == 1 and c == 3))
                                else:
                                    nc.tensor.matmul(
                                        av_ps[hh][:, ds(kt * 128, nq)],
                                        vall2[:, c, kt, sl, :],
                                        ex[:, :nq],
                                        start=(kt == 0 and c == 0),
                                        stop=(kt == 1 and c == 3))
                    # ---- normalize these 4 heads (overlaps next pass) ----
                    for hh in hds:
                        dr = 64 if hh % 2 == 0 else 0
                        nc.vector.tensor_copy(denrow[dr:dr + 1, hh, :],
                                              av_ps[hh][dr:dr + 1, :])
                    rdram = dram.tile([4, CH], FP, tag="rdram",
                                      name=f"rd{l}_{pas}")
                    for i, hh in enumerate(hds):
                        dr = 64 if hh % 2 == 0 else 0
                        nc.sync.dma_start(rdram[i:i + 1, :],
                                          denrow[dr:dr + 1, hh, :])
                    rbc2 = avacc.tile([128, 2, CH], FP, tag="rbc")
                    for j in range(2):
                        i_e = hds.index(4 * pas + 2 * j)      # even head idx
                        i_o = hds.index(4 * pas + 2 * j + 1)  # odd head idx
                        nc.sync.dma_start(
                            rbc2[0:64, j, :],
                            bass.AP(tensor=rdram.tensor,
                                    offset=rdram.offset + i_e * CH,
                                    ap=[[0, 64], [1, CH]]))
                        nc.sync.dma_start(
                            rbc2[64:128, j, :],
                            bass.AP(tensor=rdram.tensor,
                                    offset=rdram.offset + i_o * CH,
                                    ap=[[0, 64], [1, CH]]))
                    nc.vector.reciprocal_approx_fast(rbc2[:], rbc2[:])
                    for j in range(2):
                        dt_ = 2 * pas + j
                        he, ho = 4 * pas + 2 * j, 4 * pas + 2 * j + 1
                        nc.vector.tensor_tensor(attn_nb2[0:64, dt_, :],
                                                av_ps[he][0:64, :],
                                                rbc2[0:64, j, :], OP.mult)
                        nc.vector.tensor_tensor(attn_nb2[64:128, dt_, :],
                                                av_ps[ho][64:128, :],
                                                rbc2[64:128, j, :], OP.mult)

                # ---- Wo + bias + residual ----
                if with_bias:
                    bo_b = bcp.tile([128, D], FP, tag="bc")
                    bo_src = bo2[l, 0]
                    nc.sync.dma_start(bo_b[:], bass.AP(
                        tensor=bo_src.tensor, offset=bo_src.offset,
                        ap=[[0, 128]] + list(bo_src.ap)))
                for tt in range(2):
                    ps = mmp.tile([128, 512], FP, tag="mm")
                    for k in range(KD):
                        nc.tensor.matmul(ps[:, :D],
                                         attn_nb2[:, k, ds(tt * 128, 128)],
                                         wo_sb[:, k, :], start=(k == 0),
                                         stop=(k == KD - 1))
                    if with_bias:
                        nc.vector.tensor_tensor(ps[:, :D], ps[:, :D], bo_b[:],
                                                OP.add)
                    nc.vector.tensor_tensor(xt[:, tt, :], xt[:, tt, :], ps[:, :D],
                                            OP.add)

                # ---- LN2 + transpose ----
                h2 = layernorm(xt, "h")
                h2T = transpose2(h2, "t4")

                # ---- MLP ----
                if with_bias:
                    b2_b = bcp.tile([128, D], FP, tag="bc")
                    b2_src = bo2[l, 1]
                    nc.sync.dma_start(b2_b[:], bass.AP(
                        tensor=b2_src.tensor, offset=b2_src.offset,
                        ap=[[0, 128]] + list(b2_src.ap)))
                x2ps = [avp.tile([128, D], FP, tag="av", name=f"mo{l}_{kk}")
                        for kk in range(2)]
                for m in range(MD):
                    gps = mmp.tile([128, 512], FP, tag="mm")
                    for k in range(KD):
                        nc.tensor.matmul(gps[:, :CH],
                                         w1_sb[:, k, ds(m * 128, 128)],
                                         h2T[:, k, :], start=(k == 0),
                                         stop=(k == KD - 1))
                    gt = gtp.tile([128, CH], BF, tag="gt")
                    nc.scalar.activation(gt[:], gps[:, :CH], AF.Gelu,
                                         bias=b1_sb[:, l, m:m + 1])
                    for tt in range(2):
                        nc.tensor.matmul(x2ps[tt][:], gt[:, ds(tt * 128, 128)],
                                         w2_sb[:, m, :], start=(m == 0),
                                         stop=(m == MD - 1))
                for tt in range(2):
                    if with_bias:
                        nc.vector.tensor_tensor(x2ps[tt][:], x2ps[tt][:], b2_b[:],
                                                OP.add)
                    nc.vector.tensor_tensor(xt[:, tt, :], xt[:, tt, :], x2ps[tt][:],
                                            OP.add)

              if not with_head:
                  continue
              # ---- final LN + AllGather hidden ----
              xf = layernorm(xt, "h")
              xfT = transpose2(xf, "t4")
              xf_in = dram.tile([XF], BF, tag="xfin")
              nc.sync.dma_start(
                  xf_in[:].rearrange("(p a b) -> p a b", p=128, a=KD), xfT[:])
              xf_out = dram.tile([NC, XF], BF, tag="xfout", addr_space="Shared")
              nc.gpsimd.collective_compute(
                  "AllGather", OP.bypass,
                  replica_groups=[list(range(NC))],
                  ins=[xf_in[:].opt()], outs=[xf_out[:].opt()])
              xfall = []
              for cg in range(2):
                  xa = xftp.tile([128, KD, 4, CH], BF, tag="xft")
                  for c in range(4):
                      nc.sync.dma_start(
                          xa[:, :, c, :],
                          xf_out[cg * 4 + c, :].rearrange("(p a b) -> p a b",
                                                          p=128, a=KD))
                  xfall.append(xa)

              # ---- LM head: vocab-sharded, weight-stationary ----
              NWT = VS // 640
              for wt in range(NWT):
                  wl = w4pool.tile([128, KD, 640], BF, tag="w4h")
                  if head_mode != "nowlm" or wt == 0:
                      nc.sync.dma_start(
                          wl[:],
                          wlm.rearrange("(ko p) v -> p ko v", p=128)[:, :, ds(wt * 640, 640)])
                  for j in range(5):
                      vb = wt * 5 + j
                      if vb % 2 == 0:
                          pspec = [(mmp, "mm"), (mmp, "mm"), (spp, "sp"),
                                   (spp, "sp")]
                      else:
                          pspec = [(avp, "av"), (avp, "av"), (avp, "av"),
                                   (avp, "av")]
                      banks = [pool_i.tile([128, 512], FP, tag=ptag,
                                           name=f"hb{vb}_{tb}")
                               for tb, (pool_i, ptag) in enumerate(pspec)]
                      for k in range(KD):
                          for tb in range(4):
                              nc.tensor.matmul(
                                  banks[tb][:],
                                  wl[:, k, ds(j * 128, 128)],
                                  xfall[tb // 2][:, k, ds((tb % 2) * 2, 2), :],
                                  start=(k == 0), stop=(k == KD - 1))
                      lgsb = lgp.tile([128, 4, 512], BF, tag="lg",
                                      name=f"lg{vb}")
                      if head_mode != "nocopy":
                          for tb in range(4):
                              if tb % 2 == 0:
                                  nc.scalar.copy(lgsb[:, tb, :], banks[tb][:])
                              else:
                                  nc.vector.tensor_copy(lgsb[:, tb, :],
                                                        banks[tb][:])
                      if head_mode != "nodma":
                          nc.sync.dma_start(
                              logits[ds(vb * 128, 128), :], lgsb[:])
    nc.compile()
    return nc


_CACHE = {}


def _get_program(with_bias=True):
    key = ("nc", with_bias)
    if key not in _CACHE:
        _CACHE[key] = build_program(with_bias=with_bias)
    return _CACHE[key]


def _token_perm():
    """Gathered column order -> global token index."""
    P = []
    for g in range(2):
        for r in range(4):
            P.extend(range(g * T + r * 128, g * T + (r + 1) * 128))
            P.extend(range(g * T + (7 - r) * 128, g * T + (8 - r) * 128))
    return np.array(P)


def _prep_inputs(inputs):
    f = lambda k: np.asarray(inputs[k], np.float32)
    bf = ml_dtypes.bfloat16
    idx = np.asarray(inputs["idx"]).astype(np.int64)
    tok_emb, pos_emb = f("tok_emb"), f("pos_emb")
    x0 = tok_emb[idx] + pos_emb[None, :T]          # [B, T, D]
    P = _token_perm()
    x0 = x0.reshape(B * T, D)[P].reshape(NC, CH, D)

    ln1_g, ln1_b = f("ln1_g"), f("ln1_b")
    ln2_g, ln2_b = f("ln2_g"), f("ln2_b")
    Wq, bq = f("Wq"), f("bq")
    Wk, bk = f("Wk"), f("bk")
    Wv, bv = f("Wv"), f("bv")
    Wo, bo = f("Wo"), f("bo")
    W1, b1 = f("W1"), f("b1")
    W2, b2 = f("W2"), f("b2")
    lnf_g, lnf_b = f("lnf_g"), f("lnf_b")
    Wlm, blm = f("Wlm"), f("blm")

    sc = 1.0 / np.sqrt(HS)
    wqe = ln1_g[:, :, None] * Wq * sc
    bqe = (np.einsum("ld,ldm->lm", ln1_b, Wq) + bq) * sc
    wke = ln1_g[:, :, None] * Wk
    bke = np.einsum("ld,ldm->lm", ln1_b, Wk) + bk
    wve = ln1_g[:, :, None] * Wv
    bve = np.einsum("ld,ldm->lm", ln1_b, Wv) + bv
    boe = np.einsum("lm,lmd->ld", bve, Wo) + bo
    w1e = ln2_g[:, :, None] * W1
    b1e = np.einsum("ld,ldf->lf", ln2_b, W1) + b1
    wlme = lnf_g[:, None] * Wlm
    blme = lnf_b @ Wlm + blm

    bqk = np.stack([bqe, bke], axis=1)             # [L, 2, D]
    bqk = bqk.reshape(L, 2, KD, 128).transpose(3, 0, 1, 2).copy()
    b1t = b1e.reshape(L, MD, 128).transpose(2, 0, 1).copy()
    bo2 = np.stack([boe, b2], axis=1)              # [L, 2, D]

    wlmp = np.zeros((D, NC * VS), np.float32)
    wlmp[:, :V] = wlme

    # causal masks per core: [p, 0, c, q]=qA vs A_c ; [p, 1, c, q]=qB vs B_c
    masks = []
    p_ = np.arange(128)
    q_ = np.arange(128)
    for core in range(NC):
        r = core % 4
        m = np.empty((128, 2, 4, 128), np.float32)
        for c in range(4):
            kposA = c * 128 + p_[:, None]
            kposB = (7 - c) * 128 + p_[:, None]
            qposA = r * 128 + q_[None, :]
            qposB = (7 - r) * 128 + q_[None, :]
            m[:, 0, c, :] = (kposA <= qposA)
            m[:, 1, c, :] = (kposB <= qposB)
        masks.append(m.astype(bf))

    shared = dict(wq=np.ascontiguousarray(wqe.astype(bf)),
                  wk=np.ascontiguousarray(wke.astype(bf)),
                  wv=np.ascontiguousarray(wve.astype(bf)),
                  wo=np.ascontiguousarray(Wo.astype(bf)),
                  w1=np.ascontiguousarray(w1e.astype(bf)),
                  w2=np.ascontiguousarray(W2.astype(bf)),
                  bqk=bqk, b1t=b1t, bo2=np.ascontiguousarray(bo2))
    in_maps = []
    for core in range(NC):
        m = dict(shared)
        m["x0"] = np.ascontiguousarray(x0[core])
        m["msk"] = masks[core]
        m["wlm"] = np.ascontiguousarray(
            wlmp[:, core * VS:(core + 1) * VS].astype(bf))
        in_maps.append(m)
    return in_maps, blme


def _run(inputs, trace=False):
    in_maps, blme = _prep_inputs(inputs)
    with_bias = bool(np.any(in_maps[0]["bo2"]))
    nc = _get_program(with_bias=with_bias)
    res = bass_utils.run_bass_kernel_spmd(nc, in_maps, core_ids=list(range(NC)),
                                          trace=trace)
    lg = np.concatenate([np.asarray(res.results[c]["logits"], np.float32)
                         for c in range(NC)], axis=0)   # [NC*VS, B*T] permuted
    P = _token_perm()
    out = np.empty((B * T, V), np.float32)
    out[P, :] = lg[:V, :].T
    if np.any(blme):
        out = out + blme[None, :]
    return out.reshape(B, T, V).astype(np.float32), res


def kernel(**inputs) -> np.ndarray:
    out, _ = _run(inputs, trace=False)
    return out
